# revision 2
# baseline (speedup 1.0000x reference)
"""NeuralWDRC Trainium2 kernel v2: 8-core data-parallel (2 samples/core).

Per core:
  1) MLP (baseline |x|-folded form, f32r matmuls, bf16-staged gru) -> p2
     with t on partitions: p2ps [125, 64].
  2) ratio chain: softplus via Exp/Ln (single act table set), clip [1,20].
  3) interp as ONE matmul per sample: lhsT Y [34,125] built via
     transpose -> line -> overlapped-reshape -> transpose; rhs M34 [34,2560].
  4) gain per compressor via min-1 form:
       rec' = thr/env  (recip_approx_fast of Act-Abs-scaled env)
       gain = 1 - clip((1-rec')*(1-rinv_k), 0, 0.9)
     computed in bf16 TT/TS ops (DVE 2x/4x modes), scan in bf16 with f32
     decay row and 128-col halo warmup + exact stream-start fixup.
  5) v = enh*s1 + (0.1*res)*s2 (10x scale folded into normalization),
     global abs-max via partition_all_reduce + AllGather(8).
"""

import numpy as np

import concourse.bass as bass
import concourse.bacc as bacc
import concourse.mybir as mybir
import concourse.tile as tile
from concourse.bass_utils import run_bass_kernel_spmd
from concourse import bass_isa

F32 = mybir.dt.float32
F32R = mybir.dt.float32r
BF16 = mybir.dt.bfloat16
I16 = mybir.dt.int16
AF = mybir.ActivationFunctionType
OP = mybir.AluOpType

NCORES = 8
S = 2
T = 4000
TB = S * T            # 8000
NSAMP = 320000
HOP = 80
GRU_H, H1, H2 = 256, 128, 64

P = 125               # audio partitions
CH = 2560             # cols per partition
HH = CH // 2
W = 128               # scan halo warmup cols
A = W + CH            # 2688

CHK = 500
NCHK = TB // CHK      # 16
TT = 125
LAM = 2.0 ** -10

_compiled = {}


def _prep_weights(W1, b1, a1, W2, b2, a2, W3, b3):
    W1 = W1.astype(np.float64); W2 = W2.astype(np.float64)
    w3 = W3.astype(np.float64)[2]
    b1 = b1.astype(np.float64); b2 = b2.astype(np.float64)
    b3r = float(np.asarray(b3, np.float64)[2])
    a1 = float(a1); a2 = float(a2)
    c1, d1 = (1 + a1) / 2, (1 - a1) / 2
    c2, d2 = (1 + a2) / 2, (1 - a2) / 2

    A2 = c1 * (W2 @ W1)
    B2 = d1 * W2
    beta2 = b2 + c1 * (W2 @ b1)

    a3 = c2 * (A2.T @ w3)
    b3v = c2 * (B2.T @ w3)
    c3v = d2 * w3
    gamma = c2 * float(w3 @ beta2) + b3r

    A2x = np.concatenate([A2, LAM * a3[None, :]], 0)
    B2x = np.concatenate([B2, LAM * b3v[None, :]], 0)
    beta2x = np.concatenate([beta2, [1.0]])
    r3 = np.concatenate([c3v, [1.0 / LAM]])
    spb = gamma - 1.0 / LAM

    W1T = W1.T
    out = {
        "w1t0": W1T[:128], "w1t1": W1T[128:],
        "a2xt0": A2x.T[:128], "a2xt1": A2x.T[128:],
        "b2xt": B2x.T,
        "r3": r3[:, None],
        "bias1": b1[:, None],
        "bias2": beta2x[:, None],
        "spbias": np.full((P, 1), spb),
    }
    return {k: np.ascontiguousarray(v, np.float32) for k, v in out.items()}


def _interp_m3():
    """[3, 80]: ratio_i[80t+k] = sum_j M3[j,k] * ratio[t-1+j]."""
    m = np.zeros((3, HOP), np.float64)
    for k in range(HOP):
        f = (k + 0.5) / HOP - 0.5
        if k < HOP // 2:
            m[0, k] = -f
            m[1, k] = 1.0 + f
        else:
            m[1, k] = 1.0 - f
            m[2, k] = f
    return np.ascontiguousarray(m, np.float32)


def _build_nc(sim=False, dbg=False):
    nc = bacc.Bacc("TRN2", target_bir_lowering=False, debug=False,
                   num_devices=NCORES)
    grut = nc.dram_tensor("grut", [GRU_H, TB], F32, kind="ExternalInput")
    enh = nc.dram_tensor("enh", [S, NSAMP], F32, kind="ExternalInput")
    noisy = nc.dram_tensor("noisy", [S, NSAMP], F32, kind="ExternalInput")
    wnames = ["w1t0", "w1t1", "a2xt0", "a2xt1", "b2xt", "r3",
              "bias1", "bias2", "spbias", "m3d", "ident"]
    wshapes = {"w1t0": [128, 128], "w1t1": [128, 128],
               "a2xt0": [128, 65], "a2xt1": [128, 65], "b2xt": [128, 65],
               "r3": [65, 1], "bias1": [128, 1], "bias2": [65, 1],
               "spbias": [P, 1], "m3d": [35, HOP], "ident": [128, 128]}
    wdram = {n: nc.dram_tensor(n, wshapes[n], F32, kind="ExternalInput")
             for n in wnames}
    out = nc.dram_tensor("out", [S, NSAMP], F32, kind="ExternalOutput")
    dbgd = {}
    if dbg:
        for dn, dshape in [("d_rat", [P, 32]), ("d_rinv", [P, CH]),
                           ("d_m1e", [P, CH]), ("d_m1r", [P, CH]),
                           ("d_ghe", [P, A]), ("d_sve", [P, A]),
                           ("d_ghr", [P, A]), ("d_svr", [P, A]),
                           ("d_v", [P, CH]), ("d_m2e", [P, CH]),
                           ("d_enh", [P, CH]), ("d_res", [P, CH])]:
            dbgd[dn] = nc.dram_tensor(dn, dshape, F32, kind="ExternalOutput")
    cc_in = nc.dram_tensor("cc_in", [2], F32)
    cc_out = nc.dram_tensor("cc_out", [2 * NCORES], F32, addr_space="Shared")

    with tile.TileContext(nc) as tc:
        with (
            tc.tile_pool(name="wpool", bufs=1) as wpool,
            tc.tile_pool(name="mlp", bufs=3) as mlp,
            tc.tile_pool(name="small", bufs=1) as small,
            tc.tile_pool(name="scr", bufs=1) as scr,
            tc.tile_pool(name="ps", bufs=2, space="PSUM") as ps,
            tc.tile_pool(name="ps1", bufs=1, space="PSUM") as ps1,
            tc.tile_pool(name="psi", bufs=1, space="PSUM") as psi,
            tc.tile_pool(name="psi2", bufs=2, space="PSUM") as psi2,
        ):
            # ---- resident weights ----
            wsb = {}
            for n in wnames:
                t_ = wpool.tile(wshapes[n], F32, tag=n, name=f"w_{n}")
                nc.sync.dma_start(t_[:], wdram[n][:])
                wsb[n] = t_
            wsr = {}
            for n in ("w1t0", "w1t1", "a2xt0", "a2xt1", "b2xt"):
                t_ = wpool.tile(wshapes[n], BF16, tag=n + "r", name=f"wr_{n}")
                nc.vector.tensor_copy(t_[:], wsb[n][:])
                wsr[n] = t_
            sh3 = wpool.tile([35, T + 34], BF16, tag="sh3")
            m3db = wpool.tile([35, HOP], BF16, tag="m3db")
            nc.vector.tensor_copy(m3db[:], wsb["m3d"][:])

            # decay row for scans; [0, W] = 0 is the stream-start fixup
            d0a = wpool.tile([P, A], F32, tag="d0a")
            nc.gpsimd.memset(d0a[:], 0.9)
            nc.gpsimd.memset(d0a[0:1, W:W + 1], 0.0)

            # per-partition act scale/bias constants
            scl = wpool.tile([128, 2], F32, tag="scl")
            nc.gpsimd.memset(scl[:, 0:1], 1.0 / 0.3)    # enh: 1/thr
            nc.gpsimd.memset(scl[:, 1:2], 100.0)        # res': 1/0.01
            epsb = wpool.tile([128, 2], F32, tag="epsb")
            nc.gpsimd.memset(epsb[:, 0:1], 1e-8 / 0.3)
            nc.gpsimd.memset(epsb[:, 1:2], 1e-7)

            p2ps = [ps1.tile([P, 32], F32, tag="p2", name=f"p2_{i}")
                    for i in range(S)]

            def pe_touch(ap):
                pass

            vmax = small.tile([P, 2 * S], F32, tag="vmax")
            emax = small.tile([P, 2 * S], F32, tag="emax")

            # ================= gru staging (host-transposed, bf16) ========
            xt0 = wpool.tile([128, TB], BF16, tag="xt0")
            xt1 = wpool.tile([128, TB], BF16, tag="xt1")
            def gruq(q):
                qs = slice(q * (TB // 4), (q + 1) * (TB // 4))
                nc.gpsimd.dma_start(xt0[:, qs], grut[0:128, qs])
                nc.gpsimd.dma_start(xt1[:, qs], grut[128:256, qs])
            gruq(0)

            # ================= audio pre-ratio (no MLP dependency) ========
            audio = []   # per sample: dict of tiles
            for s in range(S):
                enh_t = scr.tile([P, CH], BF16, tag=f"enh{s}", name=f"enh{s}")
                res_t = scr.tile([P, CH], BF16, tag=f"res{s}", name=f"res{s}")
                nc.gpsimd.dma_start(enh_t[:],
                                    enh[s].rearrange("(p n) -> p n", p=P))
                nc.gpsimd.dma_start(res_t[:],
                                    noisy[s].rearrange("(p n) -> p n", p=P))
                if s == 0:
                    gruq(1)
                else:
                    gruq(2)
                    gruq(3)
                # res' = 0.1 * (noisy - enh)   (thr scales to 0.01)
                nc.vector.tensor_tensor(res_t[:], res_t[:], enh_t[:],
                                        op=OP.subtract)
                nc.vector.tensor_scalar(res_t[:], res_t[:], 0.1, None,
                                        op0=OP.mult)
                audio.append({"enh": enh_t, "res": res_t})

            # env -> rec -> m1 chains; half-width f32 scratch, shared tags
            for s in range(S):
                au = audio[s]
                for ci, (sig, sci) in enumerate(((au["enh"], 0),
                                                 (au["res"], 1))):
                    m1 = scr.tile([P, CH], BF16, tag=f"m1_{s}{ci}",
                                  name=f"m1_{s}{ci}")
                    # env' = |x/thr + eps'| (eps folded into bias; no bf16
                    # input can cancel it to 0 exactly)
                    envf = scr.tile([P, CH], F32, tag="envf",
                                    name=f"envf{ci}_{s}")
                    nc.scalar.activation(envf[:], sig[:], AF.Abs,
                                         scale=scl[:P, sci:sci + 1],
                                         bias=epsb[:P, sci:sci + 1])
                    if ci == 0:
                        # emax' = max(env') ; true emax = 0.3 * emax'
                        nc.vector.tensor_reduce(
                            emax[:, 2 * s:2 * s + 1], envf[:, 0:HH],
                            op=OP.max, axis=mybir.AxisListType.X)
                        nc.vector.tensor_reduce(
                            emax[:, 2 * s + 1:2 * s + 2], envf[:, HH:CH],
                            op=OP.max, axis=mybir.AxisListType.X)
                    recf = scr.tile([P, CH], F32, tag="recf",
                                    name=f"recf{ci}_{s}")
                    nc.vector.reciprocal_approx_fast(out=recf[:],
                                                     in_=envf[:])
                    # m1 = relu(1 - rec')   (bf16)
                    nc.scalar.activation(m1[:], recf[:], AF.Relu,
                                         bias=1.0, scale=-1.0)
                    au[f"m1_{ci}"] = m1

            # ================= post-ratio audio ===========================
            def sample_block(s):
                au = audio[s]
                # ---- ratio chain [125, 32] ----
                # softplus(x) = relu(x) + poly(e^-|x|); Exp/Abs/Relu all
                # live in act-table set 0 -> no table switches
                px = small.tile([P, 32], F32, tag=f"px{s}", name=f"px{s}")
                nc.vector.tensor_copy(px[:], p2ps[s][:])
                ax = small.tile([P, 32], F32, tag=f"ax{s}", name=f"ax{s}")
                nc.scalar.activation(ax[:], px[:], AF.Abs,
                                     bias=wsb["spbias"][:])
                uu = small.tile([P, 32], F32, tag=f"uu{s}", name=f"uu{s}")
                nc.scalar.activation(uu[:], ax[:], AF.Exp, scale=-1.0)
                hh = small.tile([P, 32], F32, tag=f"hh{s}", name=f"hh{s}")
                nc.vector.tensor_scalar(hh[:], uu[:], 0.11477816,
                                        -0.40741059, op0=OP.mult, op1=OP.add)
                nc.vector.tensor_tensor(hh[:], hh[:], uu[:], op=OP.mult)
                nc.vector.tensor_scalar(hh[:], hh[:], 0.98669098, None,
                                        op0=OP.add)
                nc.vector.tensor_tensor(hh[:], hh[:], uu[:], op=OP.mult)
                rat = small.tile([P, 32], F32, tag=f"rat{s}", name=f"rat{s}")
                nc.vector.tensor_scalar(rat[:], px[:],
                                        wsb["spbias"][:P, 0:1], 0.0,
                                        op0=OP.add, op1=OP.max)
                nc.vector.tensor_tensor(rat[:], rat[:], hh[:], op=OP.add)
                nc.vector.tensor_scalar(rat[:], rat[:], 1.0, 20.0,
                                        op0=OP.add, op1=OP.min)
                if dbg and s == 0:
                    nc.sync.dma_start(dbgd["d_rat"][:], rat[:])
                # ---- sh3 rows (baseline scheme): row b+j col i =
                # ratio_s(i + j - 1), clipped at stream edges ----
                ratT_ps = psi.tile([32, P], F32, tag="rT")
                pe_touch(rat)
                nc.tensor.transpose(ratT_ps[:], rat[:], wsb["ident"][:P, :P])
                ratT = small.tile([32, P], BF16, tag=f"ratT{s}",
                                  name=f"ratT{s}")
                nc.scalar.copy(ratT[:], ratT_ps[:])
                b = 32 * s
                rT = ratT[:]
                r3d = lambda ap: ap.rearrange("p (r q) -> p r q", q=P)
                nc.sync.dma_start(r3d(sh3[b:b + 1, 1:T + 1]), rT)
                nc.sync.dma_start(sh3[b:b + 1, 0:1], rT[0:1, 0:1])
                nc.sync.dma_start(r3d(sh3[b + 1:b + 2, 0:T]), rT)
                nc.sync.dma_start(sh3[b + 2:b + 3, 0:124], rT[0:1, 1:P])
                nc.sync.dma_start(
                    r3d(sh3[b + 2:b + 3, 124:124 + 31 * P]), rT[1:32, :])
                nc.sync.dma_start(sh3[b + 2:b + 3, T - 1:T],
                                  rT[31:32, P - 1:P])
                # ---- interp (group matmuls) + rinv from psum ----
                rinv = scr.tile([P, CH], F32, tag="rcr", name=f"rinv{s}")
                pe_touch(sh3[0:1, 0:1] if s == 0 else sh3[0:1, 1:2])
                for g in range(6):
                    taus = list(range(g * 6, min((g + 1) * 6, 32)))
                    rips = psi2.tile([P, 480], F32, tag="rips")
                    for ti, tau in enumerate(taus):
                        lhsT = sh3[b:b + 3, tau:tau + 32 * P:32]
                        nc.tensor.matmul(rips[:, ti * HOP:(ti + 1) * HOP],
                                         lhsT, m3db[b:b + 3, :],
                                         start=True, stop=True)
                    nwid = len(taus) * HOP
                    nc.vector.reciprocal_approx_fast(
                        out=rinv[:, g * 480:g * 480 + nwid],
                        in_=rips[:, :nwid])
                if dbg and s == 0:
                    nc.sync.dma_start(dbgd["d_rinv"][:], rinv[:])
                m2e = scr.tile([P, CH], BF16, tag=f"m2e{s}", name=f"m2e{s}")
                nc.scalar.activation(m2e[:], rinv[:], AF.Copy,
                                     bias=1.0, scale=-1.0)
                m2r = scr.tile([P, CH], BF16, tag=f"m2r{s}", name=f"m2r{s}")
                nc.vector.tensor_scalar(m2r[:], m2e[:], 2.0, -1.0,
                                        op0=OP.mult, op1=OP.add)

                if dbg and s == 0:
                    nc.gpsimd.dma_start(dbgd["d_m2e"][:], m2e[:])
                    nc.gpsimd.dma_start(dbgd["d_m1e"][:], au["m1_0"][:])
                    nc.gpsimd.dma_start(dbgd["d_m1r"][:], au["m1_1"][:])
                    nc.gpsimd.dma_start(dbgd["d_enh"][:], au["enh"][:])
                    nc.gpsimd.dma_start(dbgd["d_res"][:], au["res"][:])
                # ---- gains + scans + combine ----
                svs = []
                for ci, m2 in enumerate((m2e, m2r)):
                    m1 = au[f"m1_{ci}"]
                    gh = scr.tile([P, A], BF16, tag=f"gh{ci}",
                                  name=f"gh{s}{ci}")
                    gm = gh[:, W:A]
                    nc.vector.tensor_tensor(gm, m1[:], m2[:], op=OP.mult)
                    nc.vector.tensor_scalar(gm, gm, -1.0, 0.9,
                                            op0=OP.max, op1=OP.min)
                    nc.vector.tensor_scalar(gm, gm, -1.0, 1.0,
                                            op0=OP.mult, op1=OP.add)
                    # halo: prev partition's last W gains
                    ght = scr.tile([P, W], BF16, tag=f"ght{ci}",
                                   name=f"ght{s}{ci}")
                    nc.vector.tensor_copy(ght[:], gh[:, A - W:A])
                    nc.sync.dma_start(gh[1:P, 0:W], ght[0:P - 1, :])
                    nc.gpsimd.memset(gh[0:1, 0:W], 1.0)
                    # stream-start fixup: s[0, W] must be 10*g0
                    nc.vector.tensor_scalar(gh[0:1, W:W + 1],
                                            gh[0:1, W:W + 1], 10.0, None,
                                            op0=OP.mult)
                    sv = scr.tile([P, A], BF16, tag=f"sv{ci}",
                                  name=f"sv{s}{ci}")
                    nc.vector.tensor_tensor_scan(sv[:], d0a[:], gh[:], 0.0,
                                                 op0=OP.mult, op1=OP.add)
                    if dbg and s == 0:
                        nm = "e" if ci == 0 else "r"
                        nc.gpsimd.dma_start(dbgd["d_gh" + nm][:], gh[:])
                        nc.gpsimd.dma_start(dbgd["d_sv" + nm][:], sv[:])
                    svs.append(sv)

                # v = enh*s1 + res'*s2 (both scans carry 10x; folded later)
                ce = scr.tile([P, CH], BF16, tag=f"ce{s}", name=f"ce{s}")
                nc.vector.tensor_tensor(ce[:], au["enh"][:], svs[0][:, W:A],
                                        op=OP.mult)
                cr = scr.tile([P, CH], BF16, tag="rcr", name=f"cr{s}")
                nc.vector.tensor_tensor(cr[:], au["res"][:], svs[1][:, W:A],
                                        op=OP.mult)
                nc.vector.tensor_tensor(ce[:], ce[:], cr[:], op=OP.add)
                nc.vector.tensor_reduce(vmax[:, 2 * s:2 * s + 1],
                                        ce[:, 0:HH], op=OP.max,
                                        axis=mybir.AxisListType.X,
                                        apply_absolute_value=True)
                nc.vector.tensor_reduce(vmax[:, 2 * s + 1:2 * s + 2],
                                        ce[:, HH:CH], op=OP.max,
                                        axis=mybir.AxisListType.X,
                                        apply_absolute_value=True)
                if dbg and s == 0:
                    nc.gpsimd.dma_start(dbgd["d_v"][:], ce[:])
                au["v"] = ce

            # ================= MLP chunks =================================
            for c in range(NCHK):
                cs = slice(c * CHK, (c + 1) * CHK)
                x0 = xt0[:, cs]
                x1 = xt1[:, cs]

                yps = ps.tile([128, CHK], F32, tag="yz")
                nc.tensor.matmul(yps[:], wsr["w1t0"][:], x0,
                                 start=True, stop=False)
                nc.tensor.matmul(yps[:], wsr["w1t1"][:], x1,
                                 start=False, stop=True)
                ay = mlp.tile([128, CHK], BF16, tag="ay")
                nc.scalar.activation(ay[:], yps[:], AF.Abs,
                                     bias=wsb["bias1"][:])

                zfull = ps.tile([128, CHK], F32, tag="zz")
                zps = zfull[0:65, :]
                nc.tensor.matmul(zps[:], wsr["a2xt0"][:], x0,
                                 start=True, stop=False)
                nc.tensor.matmul(zps[:], wsr["a2xt1"][:], x1,
                                 start=False, stop=False)
                nc.tensor.matmul(zps[:], wsr["b2xt"][:], ay[:],
                                 start=False, stop=True)
                t2 = mlp.tile([65, CHK], F32, tag="t2")
                nc.scalar.activation(t2[:], zps[:], AF.Abs,
                                     bias=wsb["bias2"][:])

                for j in range(CHK // TT):
                    cc = c * (CHK // TT) + j
                    nc.tensor.matmul(p2ps[cc // 32][:, cc % 32:cc % 32 + 1],
                                     t2[:, j * TT:(j + 1) * TT],
                                     wsb["r3"][:], start=True, stop=True)
                if c == NCHK // 2 - 1:
                    sample_block(0)
                elif c == NCHK - 1:
                    sample_block(1)

            # ================= global normalization =======================
            gmax = small.tile([P, 2], F32, tag="gmax")
            nc.vector.tensor_reduce(gmax[:, 0:1], vmax[:], op=OP.max,
                                    axis=mybir.AxisListType.X)
            nc.vector.tensor_reduce(gmax[:, 1:2], emax[:], op=OP.max,
                                    axis=mybir.AxisListType.X)
            gmr = small.tile([P, 2], F32, tag="gmr")
            nc.gpsimd.partition_all_reduce(gmr[:], gmax[:], channels=P,
                                           reduce_op=bass_isa.ReduceOp.max)
            ccsb = small.tile([1, 2 * NCORES], F32, tag="ccsb")
            if sim:
                gbc = small.tile([NCORES, 2], F32, tag="gbc")
                nc.gpsimd.partition_broadcast(gbc[:], gmr[0:1, 0:2],
                                              channels=NCORES)
                nc.sync.dma_start(ccsb[0:1, 0:NCORES],
                                  gbc[0:NCORES // 2, :].rearrange(
                                      "p r -> (p r)")[None, :])
                nc.sync.dma_start(ccsb[0:1, NCORES:2 * NCORES],
                                  gbc[NCORES // 2:NCORES, :].rearrange(
                                      "p r -> (p r)")[None, :])
            else:
                with tc.tile_critical():
                    cc_sem = nc.alloc_semaphore("ccs")
                    nc.gpsimd.dma_start(cc_in[:], gmr[0:1, 0:2]).then_inc(
                        cc_sem, 16)
                    nc.gpsimd.collective_compute(
                        "AllGather", OP.bypass,
                        replica_groups=[list(range(NCORES))],
                        ins=[cc_in[:]], outs=[cc_out[:]],
                    )._wait_ge(cc_sem, 16).then_inc(cc_sem, 1)
                    nc.gpsimd.dma_start(ccsb[:], cc_out[None, :])._wait_ge(
                        cc_sem, 17).then_inc(cc_sem, 16)
                    nc.gpsimd.engine_nop()._wait_ge(cc_sem, 33)

            sg = small.tile([1, 4], F32, tag="sg")
            nc.vector.tensor_reduce(sg[:, 0:1], ccsb[:, 0:2 * NCORES:2],
                                    op=OP.max, axis=mybir.AxisListType.X)
            nc.vector.tensor_reduce(sg[:, 1:2], ccsb[:, 1:2 * NCORES:2],
                                    op=OP.max, axis=mybir.AxisListType.X)
            # sigma = 0.3*emax' / (vmax + 1e-7)  (v is 10x; 0.1 folded)
            nc.vector.tensor_scalar(sg[:, 2:3], sg[:, 0:1], 1e-7, None,
                                    op0=OP.add)
            nc.vector.reciprocal_approx_fast(out=sg[:, 0:1], in_=sg[:, 2:3])
            nc.vector.tensor_scalar(sg[:, 1:2], sg[:, 1:2], 0.3, None,
                                    op0=OP.mult)
            nc.vector.tensor_tensor(sg[:, 3:4], sg[:, 0:1], sg[:, 1:2],
                                    op=OP.mult)
            sgb = small.tile([P, 1], F32, tag="sgb")
            nc.gpsimd.partition_broadcast(sgb[:], sg[0:1, 3:4], channels=P)

            for s in range(S):
                v = audio[s]["v"]
                nc.vector.tensor_scalar(v[:], v[:], sgb[:, 0:1], None,
                                        op0=OP.mult)
                nc.gpsimd.dma_start(out[s].rearrange("(p n) -> p n", p=P),
                                    v[:])
    nc.finalize()
    return nc


def kernel(trace=False, **inputs):
    gru = np.ascontiguousarray(np.asarray(inputs["gru_output"], np.float32))
    enh = np.ascontiguousarray(np.asarray(inputs["enhanced"], np.float32))
    noisy = np.ascontiguousarray(np.asarray(inputs["noisy"], np.float32))
    B = gru.shape[0]
    wts = _prep_weights(inputs["W1"], inputs["b1"], inputs["a1"],
                        inputs["W2"], inputs["b2"], inputs["a2"],
                        inputs["W3"], inputs["b3"])
    m3 = _interp_m3()
    m3d = np.zeros((35, HOP), np.float32)
    m3d[0:3] = m3
    m3d[32:35] = m3
    wts["m3d"] = m3d
    wts["ident"] = np.ascontiguousarray(np.eye(128, dtype=np.float32))

    if "nc" not in _compiled:
        _compiled["nc"] = _build_nc()
    nc = _compiled["nc"]

    per = B // NCORES
    in_maps = []
    for c in range(NCORES):
        m = {
            "grut": np.ascontiguousarray(
                gru[c * per:(c + 1) * per].reshape(TB, GRU_H).T),
            "enh": np.ascontiguousarray(enh[c * per:(c + 1) * per]),
            "noisy": np.ascontiguousarray(noisy[c * per:(c + 1) * per]),
        }
        m.update(wts)
        in_maps.append(m)

    res = run_bass_kernel_spmd(nc, in_maps, list(range(NCORES)), trace=trace)
    outs = [res.results[c]["out"] for c in range(NCORES)]
    full = np.concatenate(outs, axis=0)
    if trace:
        return full, res
    return full


if __name__ == "__main__":
    pass


# revision 3
# speedup vs baseline: 1.0017x; 1.0017x over previous
"""NeuralWDRC Trainium2 kernel v2: 8-core data-parallel (2 samples/core).

Per core:
  1) MLP (baseline |x|-folded form, f32r matmuls, bf16-staged gru) -> p2
     with t on partitions: p2ps [125, 64].
  2) ratio chain: softplus via Exp/Ln (single act table set), clip [1,20].
  3) interp as ONE matmul per sample: lhsT Y [34,125] built via
     transpose -> line -> overlapped-reshape -> transpose; rhs M34 [34,2560].
  4) gain per compressor via min-1 form:
       rec' = thr/env  (recip_approx_fast of Act-Abs-scaled env)
       gain = 1 - clip((1-rec')*(1-rinv_k), 0, 0.9)
     computed in bf16 TT/TS ops (DVE 2x/4x modes), scan in bf16 with f32
     decay row and 128-col halo warmup + exact stream-start fixup.
  5) v = enh*s1 + (0.1*res)*s2 (10x scale folded into normalization),
     global abs-max via partition_all_reduce + AllGather(8).
"""

import numpy as np

import concourse.bass as bass
import concourse.bacc as bacc
import concourse.mybir as mybir
import concourse.tile as tile
from concourse.bass_utils import run_bass_kernel_spmd
from concourse import bass_isa

F32 = mybir.dt.float32
F32R = mybir.dt.float32r
BF16 = mybir.dt.bfloat16
I16 = mybir.dt.int16
AF = mybir.ActivationFunctionType
OP = mybir.AluOpType

NCORES = 8
S = 2
T = 4000
TB = S * T            # 8000
NSAMP = 320000
HOP = 80
GRU_H, H1, H2 = 256, 128, 64

P = 125               # audio partitions
CH = 2560             # cols per partition
HH = CH // 2
W = 128               # scan halo warmup cols
A = W + CH            # 2688

CHK = 500
NCHK = TB // CHK      # 16
TT = 125
LAM = 2.0 ** -10

_compiled = {}


def _prep_weights(W1, b1, a1, W2, b2, a2, W3, b3):
    W1 = W1.astype(np.float64); W2 = W2.astype(np.float64)
    w3 = W3.astype(np.float64)[2]
    b1 = b1.astype(np.float64); b2 = b2.astype(np.float64)
    b3r = float(np.asarray(b3, np.float64)[2])
    a1 = float(a1); a2 = float(a2)
    c1, d1 = (1 + a1) / 2, (1 - a1) / 2
    c2, d2 = (1 + a2) / 2, (1 - a2) / 2

    A2 = c1 * (W2 @ W1)
    B2 = d1 * W2
    beta2 = b2 + c1 * (W2 @ b1)

    a3 = c2 * (A2.T @ w3)
    b3v = c2 * (B2.T @ w3)
    c3v = d2 * w3
    gamma = c2 * float(w3 @ beta2) + b3r

    A2x = np.concatenate([A2, LAM * a3[None, :]], 0)
    B2x = np.concatenate([B2, LAM * b3v[None, :]], 0)
    beta2x = np.concatenate([beta2, [1.0]])
    r3 = np.concatenate([c3v, [1.0 / LAM]])
    spb = gamma - 1.0 / LAM

    W1T = W1.T
    out = {
        "w1t0": W1T[:128], "w1t1": W1T[128:],
        "a2xt0": A2x.T[:128], "a2xt1": A2x.T[128:],
        "b2xt": B2x.T,
        "r3": r3[:, None],
        "bias1": b1[:, None],
        "bias2": beta2x[:, None],
        "spbias": np.full((P, 1), spb),
    }
    return {k: np.ascontiguousarray(v, np.float32) for k, v in out.items()}


def _interp_m3():
    """[3, 80]: ratio_i[80t+k] = sum_j M3[j,k] * ratio[t-1+j]."""
    m = np.zeros((3, HOP), np.float64)
    for k in range(HOP):
        f = (k + 0.5) / HOP - 0.5
        if k < HOP // 2:
            m[0, k] = -f
            m[1, k] = 1.0 + f
        else:
            m[1, k] = 1.0 - f
            m[2, k] = f
    return np.ascontiguousarray(m, np.float32)


def _build_nc(sim=False, dbg=False):
    nc = bacc.Bacc("TRN2", target_bir_lowering=False, debug=False,
                   num_devices=NCORES)
    grut = nc.dram_tensor("grut", [GRU_H, TB], F32, kind="ExternalInput")
    enh = nc.dram_tensor("enh", [S, NSAMP], F32, kind="ExternalInput")
    noisy = nc.dram_tensor("noisy", [S, NSAMP], F32, kind="ExternalInput")
    wnames = ["w1t0", "w1t1", "a2xt0", "a2xt1", "b2xt", "r3",
              "bias1", "bias2", "spbias", "m3d", "ident"]
    wshapes = {"w1t0": [128, 128], "w1t1": [128, 128],
               "a2xt0": [128, 65], "a2xt1": [128, 65], "b2xt": [128, 65],
               "r3": [65, 1], "bias1": [128, 1], "bias2": [65, 1],
               "spbias": [P, 1], "m3d": [35, HOP], "ident": [128, 128]}
    wdram = {n: nc.dram_tensor(n, wshapes[n], F32, kind="ExternalInput")
             for n in wnames}
    out = nc.dram_tensor("out", [S, NSAMP], F32, kind="ExternalOutput")
    dbgd = {}
    if dbg:
        for dn, dshape in [("d_rat", [P, 32]), ("d_rinv", [P, CH]),
                           ("d_m1e", [P, CH]), ("d_m1r", [P, CH]),
                           ("d_ghe", [P, A]), ("d_sve", [P, A]),
                           ("d_ghr", [P, A]), ("d_svr", [P, A]),
                           ("d_v", [P, CH]), ("d_m2e", [P, CH]),
                           ("d_enh", [P, CH]), ("d_res", [P, CH])]:
            dbgd[dn] = nc.dram_tensor(dn, dshape, F32, kind="ExternalOutput")
    cc_in = nc.dram_tensor("cc_in", [2], F32)
    cc_out = nc.dram_tensor("cc_out", [2 * NCORES], F32, addr_space="Shared")

    with tile.TileContext(nc) as tc:
        with (
            tc.tile_pool(name="wpool", bufs=1) as wpool,
            tc.tile_pool(name="mlp", bufs=2) as mlp,
            tc.tile_pool(name="small", bufs=1) as small,
            tc.tile_pool(name="scr", bufs=1) as scr,
            tc.tile_pool(name="ps", bufs=2, space="PSUM") as ps,
            tc.tile_pool(name="ps1", bufs=1, space="PSUM") as ps1,
            tc.tile_pool(name="psi", bufs=1, space="PSUM") as psi,
            tc.tile_pool(name="psi2", bufs=2, space="PSUM") as psi2,
        ):
            # ---- resident weights ----
            wsb = {}
            for n in wnames:
                t_ = wpool.tile(wshapes[n], F32, tag=n, name=f"w_{n}")
                nc.sync.dma_start(t_[:], wdram[n][:])
                wsb[n] = t_
            wsr = {}
            for n in ("w1t0", "w1t1", "a2xt0", "a2xt1", "b2xt"):
                t_ = wpool.tile(wshapes[n], BF16, tag=n + "r", name=f"wr_{n}")
                nc.vector.tensor_copy(t_[:], wsb[n][:])
                wsr[n] = t_
            sh3 = wpool.tile([35, T + 34], BF16, tag="sh3")
            m3db = wpool.tile([35, HOP], BF16, tag="m3db")
            nc.vector.tensor_copy(m3db[:], wsb["m3d"][:])

            # decay row for scans; [0, W] = 0 is the stream-start fixup
            d0a = wpool.tile([P, A], F32, tag="d0a")
            nc.gpsimd.memset(d0a[:], 0.9)
            nc.gpsimd.memset(d0a[0:1, W:W + 1], 0.0)

            # per-partition act scale/bias constants
            scl = wpool.tile([128, 2], F32, tag="scl")
            nc.gpsimd.memset(scl[:, 0:1], 1.0 / 0.3)    # enh: 1/thr
            nc.gpsimd.memset(scl[:, 1:2], 100.0)        # res': 1/0.01
            epsb = wpool.tile([128, 2], F32, tag="epsb")
            nc.gpsimd.memset(epsb[:, 0:1], 1e-8 / 0.3)
            nc.gpsimd.memset(epsb[:, 1:2], 1e-7)

            p2ps = [ps1.tile([P, 32], F32, tag="p2", name=f"p2_{i}")
                    for i in range(S)]

            def pe_touch(ap):
                pass

            vemax = small.tile([P, 4 * S], F32, tag="vemax")

            # ================= gru staging (host-transposed, bf16) ========
            xt0 = wpool.tile([128, TB], BF16, tag="xt0")
            xt1 = wpool.tile([128, TB], BF16, tag="xt1")
            def gruq(q):
                qs = slice(q * (TB // 4), (q + 1) * (TB // 4))
                nc.gpsimd.dma_start(xt0[:, qs], grut[0:128, qs])
                nc.gpsimd.dma_start(xt1[:, qs], grut[128:256, qs])
            gruq(0)

            # ================= audio pre-ratio (no MLP dependency) ========
            audio = []   # per sample: dict of tiles
            for s in range(S):
                enh_t = scr.tile([P, CH], BF16, tag=f"enh{s}", name=f"enh{s}")
                res_t = scr.tile([P, CH], BF16, tag=f"res{s}", name=f"res{s}")
                nc.gpsimd.dma_start(enh_t[:],
                                    enh[s].rearrange("(p n) -> p n", p=P))
                nc.gpsimd.dma_start(res_t[:],
                                    noisy[s].rearrange("(p n) -> p n", p=P))
                if s == 0:
                    gruq(1)
                else:
                    gruq(2)
                    gruq(3)
                # res' = 0.1 * (noisy - enh)   (thr scales to 0.01)
                nc.vector.tensor_tensor(res_t[:], res_t[:], enh_t[:],
                                        op=OP.subtract)
                nc.vector.tensor_scalar(res_t[:], res_t[:], 0.1, None,
                                        op0=OP.mult)
                audio.append({"enh": enh_t, "res": res_t})

            # env -> rec -> m1 chains
            def env_chains(s):
                au = audio[s]
                for ci, (sig, sci) in enumerate(((au["enh"], 0),
                                                 (au["res"], 1))):
                    m1 = scr.tile([P, CH], BF16, tag=f"m1_{s}{ci}",
                                  name=f"m1_{s}{ci}")
                    # env' = |x/thr + eps'| (eps folded into bias; no bf16
                    # input can cancel it to 0 exactly)
                    envf = scr.tile([P, CH], F32, tag=f"envf{ci}",
                                    name=f"envf{ci}_{s}")
                    nc.scalar.activation(envf[:], sig[:], AF.Abs,
                                         scale=scl[:P, sci:sci + 1],
                                         bias=epsb[:P, sci:sci + 1])
                    if ci == 0:
                        # emax' = max(env') ; true emax = 0.3 * emax'
                        nc.vector.tensor_reduce(
                            vemax[:, 2 * S + 2 * s:2 * S + 2 * s + 1],
                            envf[:, 0:HH],
                            op=OP.max, axis=mybir.AxisListType.X)
                        nc.vector.tensor_reduce(
                            vemax[:, 2 * S + 2 * s + 1:2 * S + 2 * s + 2],
                            envf[:, HH:CH],
                            op=OP.max, axis=mybir.AxisListType.X)
                    recf = scr.tile([P, CH], F32, tag=f"recf{ci}",
                                    name=f"recf{ci}_{s}")
                    nc.vector.reciprocal_approx_fast(out=recf[:],
                                                     in_=envf[:])
                    # m1 = relu(1 - rec')   (bf16)
                    nc.scalar.activation(m1[:], recf[:], AF.Relu,
                                         bias=1.0, scale=-1.0)
                    au[f"m1_{ci}"] = m1

            env_chains(0)
            env_chains(1)

            # ================= post-ratio audio ===========================
            def sample_block(s):
                au = audio[s]
                # ---- ratio chain [125, 32] ----
                # softplus(x) = relu(x) + poly(e^-|x|); Exp/Abs/Relu all
                # live in act-table set 0 -> no table switches
                px = small.tile([P, 32], F32, tag=f"px{s}", name=f"px{s}")
                nc.vector.tensor_copy(px[:], p2ps[s][:])
                ax = small.tile([P, 32], F32, tag=f"ax{s}", name=f"ax{s}")
                nc.scalar.activation(ax[:], px[:], AF.Abs,
                                     bias=wsb["spbias"][:])
                uu = small.tile([P, 32], F32, tag=f"uu{s}", name=f"uu{s}")
                nc.scalar.activation(uu[:], ax[:], AF.Exp, scale=-1.0)
                hh = small.tile([P, 32], F32, tag=f"hh{s}", name=f"hh{s}")
                nc.vector.tensor_scalar(hh[:], uu[:], 0.11477816,
                                        -0.40741059, op0=OP.mult, op1=OP.add)
                nc.vector.tensor_tensor(hh[:], hh[:], uu[:], op=OP.mult)
                nc.vector.tensor_scalar(hh[:], hh[:], 0.98669098, None,
                                        op0=OP.add)
                nc.vector.tensor_tensor(hh[:], hh[:], uu[:], op=OP.mult)
                rat = small.tile([P, 32], F32, tag=f"rat{s}", name=f"rat{s}")
                nc.vector.tensor_scalar(rat[:], px[:],
                                        wsb["spbias"][:P, 0:1], 0.0,
                                        op0=OP.add, op1=OP.max)
                nc.vector.tensor_tensor(rat[:], rat[:], hh[:], op=OP.add)
                nc.vector.tensor_scalar(rat[:], rat[:], 1.0, 20.0,
                                        op0=OP.add, op1=OP.min)
                if dbg and s == 0:
                    nc.sync.dma_start(dbgd["d_rat"][:], rat[:])
                # ---- sh3 rows (baseline scheme): row b+j col i =
                # ratio_s(i + j - 1), clipped at stream edges ----
                ratT_ps = psi.tile([32, P], F32, tag="rT")
                pe_touch(rat)
                nc.tensor.transpose(ratT_ps[:], rat[:], wsb["ident"][:P, :P])
                ratT = small.tile([32, P], BF16, tag=f"ratT{s}",
                                  name=f"ratT{s}")
                nc.scalar.copy(ratT[:], ratT_ps[:])
                b = 32 * s
                rT = ratT[:]
                r3d = lambda ap: ap.rearrange("p (r q) -> p r q", q=P)
                nc.sync.dma_start(r3d(sh3[b:b + 1, 1:T + 1]), rT)
                nc.sync.dma_start(sh3[b:b + 1, 0:1], rT[0:1, 0:1])
                nc.sync.dma_start(r3d(sh3[b + 1:b + 2, 0:T]), rT)
                nc.sync.dma_start(sh3[b + 2:b + 3, 0:124], rT[0:1, 1:P])
                nc.sync.dma_start(
                    r3d(sh3[b + 2:b + 3, 124:124 + 31 * P]), rT[1:32, :])
                nc.sync.dma_start(sh3[b + 2:b + 3, T - 1:T],
                                  rT[31:32, P - 1:P])
                # ---- interp (group matmuls) + rinv from psum ----
                rinv = scr.tile([P, CH], F32, tag="rcr", name=f"rinv{s}")
                pe_touch(sh3[0:1, 0:1] if s == 0 else sh3[0:1, 1:2])
                for g in range(6):
                    taus = list(range(g * 6, min((g + 1) * 6, 32)))
                    rips = psi2.tile([P, 480], F32, tag="rips")
                    for ti, tau in enumerate(taus):
                        lhsT = sh3[b:b + 3, tau:tau + 32 * P:32]
                        nc.tensor.matmul(rips[:, ti * HOP:(ti + 1) * HOP],
                                         lhsT, m3db[b:b + 3, :],
                                         start=True, stop=True)
                    nwid = len(taus) * HOP
                    nc.vector.reciprocal_approx_fast(
                        out=rinv[:, g * 480:g * 480 + nwid],
                        in_=rips[:, :nwid])
                if dbg and s == 0:
                    nc.sync.dma_start(dbgd["d_rinv"][:], rinv[:])
                m2e = scr.tile([P, CH], BF16, tag=f"m2e{s}", name=f"m2e{s}")
                nc.scalar.activation(m2e[:], rinv[:], AF.Copy,
                                     bias=1.0, scale=-1.0)
                m2r = scr.tile([P, CH], BF16, tag=f"m2r{s}", name=f"m2r{s}")
                nc.scalar.activation(m2r[:], m2e[:], AF.Copy,
                                     bias=-1.0, scale=2.0)

                if dbg and s == 0:
                    nc.gpsimd.dma_start(dbgd["d_m2e"][:], m2e[:])
                    nc.gpsimd.dma_start(dbgd["d_m1e"][:], au["m1_0"][:])
                    nc.gpsimd.dma_start(dbgd["d_m1r"][:], au["m1_1"][:])
                    nc.gpsimd.dma_start(dbgd["d_enh"][:], au["enh"][:])
                    nc.gpsimd.dma_start(dbgd["d_res"][:], au["res"][:])
                # ---- gains + scans + combine ----
                svs = []
                for ci, m2 in enumerate((m2e, m2r)):
                    m1 = au[f"m1_{ci}"]
                    gh = scr.tile([P, A], BF16, tag=f"gh{ci}",
                                  name=f"gh{s}{ci}")
                    gm = gh[:, W:A]
                    nc.vector.tensor_tensor(gm, m1[:], m2[:], op=OP.mult)
                    nc.vector.tensor_scalar(gm, gm, -1.0, 0.9,
                                            op0=OP.max, op1=OP.min)
                    nc.scalar.activation(gm, gm, AF.Copy,
                                         bias=1.0, scale=-1.0)
                    # halo: prev partition's last W gains
                    ght = scr.tile([P, W], BF16, tag=f"ght{ci}",
                                   name=f"ght{s}{ci}")
                    nc.vector.tensor_copy(ght[:], gh[:, A - W:A])
                    nc.sync.dma_start(gh[1:P, 0:W], ght[0:P - 1, :])
                    nc.gpsimd.memset(gh[0:1, 0:W], 1.0)
                    # stream-start fixup: s[0, W] must be 10*g0
                    nc.vector.tensor_scalar(gh[0:1, W:W + 1],
                                            gh[0:1, W:W + 1], 10.0, None,
                                            op0=OP.mult)
                    sv = scr.tile([P, A], BF16, tag=f"sv{ci}",
                                  name=f"sv{s}{ci}")
                    nc.vector.tensor_tensor_scan(sv[:], d0a[:], gh[:], 0.0,
                                                 op0=OP.mult, op1=OP.add)
                    if dbg and s == 0:
                        nm = "e" if ci == 0 else "r"
                        nc.gpsimd.dma_start(dbgd["d_gh" + nm][:], gh[:])
                        nc.gpsimd.dma_start(dbgd["d_sv" + nm][:], sv[:])
                    svs.append(sv)

                # v = enh*s1 + res'*s2 (both scans carry 10x; folded later)
                ce = scr.tile([P, CH], BF16, tag=f"ce{s}", name=f"ce{s}")
                nc.vector.tensor_tensor(ce[:], au["enh"][:], svs[0][:, W:A],
                                        op=OP.mult)
                cr = scr.tile([P, CH], BF16, tag="rcr", name=f"cr{s}")
                nc.vector.tensor_tensor(cr[:], au["res"][:], svs[1][:, W:A],
                                        op=OP.mult)
                nc.vector.tensor_tensor(ce[:], ce[:], cr[:], op=OP.add)
                for h in range(2):
                    hs = slice(HH * h, HH * (h + 1))
                    nc.vector.tensor_reduce(
                        vemax[:, 2 * s + h:2 * s + h + 1], ce[:, hs],
                        op=OP.max, axis=mybir.AxisListType.X,
                        apply_absolute_value=True)
                if dbg and s == 0:
                    nc.gpsimd.dma_start(dbgd["d_v"][:], ce[:])
                au["v"] = ce

            # ================= MLP chunks =================================
            for c in range(NCHK):
                cs = slice(c * CHK, (c + 1) * CHK)
                x0 = xt0[:, cs]
                x1 = xt1[:, cs]

                yps = ps.tile([128, CHK], F32, tag="yz")
                nc.tensor.matmul(yps[:], wsr["w1t0"][:], x0,
                                 start=True, stop=False)
                nc.tensor.matmul(yps[:], wsr["w1t1"][:], x1,
                                 start=False, stop=True)
                ay = mlp.tile([128, CHK], BF16, tag="ay")
                nc.scalar.activation(ay[:], yps[:], AF.Abs,
                                     bias=wsb["bias1"][:])

                zfull = ps.tile([128, CHK], F32, tag="zz")
                zps = zfull[0:65, :]
                nc.tensor.matmul(zps[:], wsr["a2xt0"][:], x0,
                                 start=True, stop=False)
                nc.tensor.matmul(zps[:], wsr["a2xt1"][:], x1,
                                 start=False, stop=False)
                nc.tensor.matmul(zps[:], wsr["b2xt"][:], ay[:],
                                 start=False, stop=True)
                t2 = mlp.tile([65, CHK], F32, tag="t2")
                nc.scalar.activation(t2[:], zps[:], AF.Abs,
                                     bias=wsb["bias2"][:])

                for j in range(CHK // TT):
                    cc = c * (CHK // TT) + j
                    nc.tensor.matmul(p2ps[cc // 32][:, cc % 32:cc % 32 + 1],
                                     t2[:, j * TT:(j + 1) * TT],
                                     wsb["r3"][:], start=True, stop=True)
                if c == NCHK // 2 - 1:
                    sample_block(0)
                elif c == NCHK - 1:
                    sample_block(1)

            # ================= global normalization =======================
            galr = small.tile([P, 4 * S], F32, tag="galr")
            nc.gpsimd.partition_all_reduce(galr[:], vemax[:], channels=P,
                                           reduce_op=bass_isa.ReduceOp.max)
            gmr = small.tile([P, 2], F32, tag="gmr")
            nc.vector.tensor_reduce(gmr[0:1, 0:1], galr[0:1, 0:2 * S],
                                    op=OP.max, axis=mybir.AxisListType.X)
            nc.vector.tensor_reduce(gmr[0:1, 1:2],
                                    galr[0:1, 2 * S:4 * S],
                                    op=OP.max, axis=mybir.AxisListType.X)
            ccsb = small.tile([1, 2 * NCORES], F32, tag="ccsb")
            if sim:
                gbc = small.tile([NCORES, 2], F32, tag="gbc")
                nc.gpsimd.partition_broadcast(gbc[:], gmr[0:1, 0:2],
                                              channels=NCORES)
                nc.sync.dma_start(ccsb[0:1, 0:NCORES],
                                  gbc[0:NCORES // 2, :].rearrange(
                                      "p r -> (p r)")[None, :])
                nc.sync.dma_start(ccsb[0:1, NCORES:2 * NCORES],
                                  gbc[NCORES // 2:NCORES, :].rearrange(
                                      "p r -> (p r)")[None, :])
            else:
                with tc.tile_critical():
                    cc_sem = nc.alloc_semaphore("ccs")
                    nc.gpsimd.dma_start(cc_in[:], gmr[0:1, 0:2]).then_inc(
                        cc_sem, 16)
                    nc.gpsimd.collective_compute(
                        "AllGather", OP.bypass,
                        replica_groups=[list(range(NCORES))],
                        ins=[cc_in[:]], outs=[cc_out[:]],
                    )._wait_ge(cc_sem, 16).then_inc(cc_sem, 1)
                    nc.gpsimd.dma_start(ccsb[:], cc_out[None, :])._wait_ge(
                        cc_sem, 17).then_inc(cc_sem, 16)
                    nc.gpsimd.engine_nop()._wait_ge(cc_sem, 33)

            sg = small.tile([1, 4], F32, tag="sg")
            nc.vector.tensor_reduce(sg[:, 0:1], ccsb[:, 0:2 * NCORES:2],
                                    op=OP.max, axis=mybir.AxisListType.X)
            nc.vector.tensor_reduce(sg[:, 1:2], ccsb[:, 1:2 * NCORES:2],
                                    op=OP.max, axis=mybir.AxisListType.X)
            # sigma = 0.3*emax' / (vmax + 1e-7)  (v is 10x; 0.1 folded)
            nc.vector.tensor_scalar(sg[:, 2:3], sg[:, 0:1], 1e-7, None,
                                    op0=OP.add)
            nc.vector.reciprocal_approx_fast(out=sg[:, 0:1], in_=sg[:, 2:3])
            nc.vector.tensor_scalar(sg[:, 1:2], sg[:, 1:2], 0.3, None,
                                    op0=OP.mult)
            nc.vector.tensor_tensor(sg[:, 3:4], sg[:, 0:1], sg[:, 1:2],
                                    op=OP.mult)
            sgb = small.tile([P, 1], F32, tag="sgb")
            nc.gpsimd.partition_broadcast(sgb[:], sg[0:1, 3:4], channels=P)

            for s in range(S):
                v = audio[s]["v"]
                ov = out[s].rearrange("(p n) -> p n", p=P)
                for h in range(2):
                    hs = slice(HH * h, HH * (h + 1))
                    nc.vector.tensor_scalar(v[:, hs], v[:, hs],
                                            sgb[:, 0:1], None, op0=OP.mult)
                    nc.gpsimd.dma_start(ov[:, hs], v[:, hs])
    nc.finalize()
    return nc


def kernel(trace=False, **inputs):
    gru = np.ascontiguousarray(np.asarray(inputs["gru_output"], np.float32))
    enh = np.ascontiguousarray(np.asarray(inputs["enhanced"], np.float32))
    noisy = np.ascontiguousarray(np.asarray(inputs["noisy"], np.float32))
    B = gru.shape[0]
    wts = _prep_weights(inputs["W1"], inputs["b1"], inputs["a1"],
                        inputs["W2"], inputs["b2"], inputs["a2"],
                        inputs["W3"], inputs["b3"])
    m3 = _interp_m3()
    m3d = np.zeros((35, HOP), np.float32)
    m3d[0:3] = m3
    m3d[32:35] = m3
    wts["m3d"] = m3d
    wts["ident"] = np.ascontiguousarray(np.eye(128, dtype=np.float32))

    if "nc" not in _compiled:
        _compiled["nc"] = _build_nc()
    nc = _compiled["nc"]

    per = B // NCORES
    in_maps = []
    for c in range(NCORES):
        m = {
            "grut": np.ascontiguousarray(
                gru[c * per:(c + 1) * per].reshape(TB, GRU_H).T),
            "enh": np.ascontiguousarray(enh[c * per:(c + 1) * per]),
            "noisy": np.ascontiguousarray(noisy[c * per:(c + 1) * per]),
        }
        m.update(wts)
        in_maps.append(m)

    res = run_bass_kernel_spmd(nc, in_maps, list(range(NCORES)), trace=trace)
    outs = [res.results[c]["out"] for c in range(NCORES)]
    full = np.concatenate(outs, axis=0)
    if trace:
        return full, res
    return full


if __name__ == "__main__":
    pass


# revision 4
# speedup vs baseline: 1.0183x; 1.0167x over previous
"""NeuralWDRC Trainium2 kernel v2: 8-core data-parallel (2 samples/core).

Per core:
  1) MLP (baseline |x|-folded form, f32r matmuls, bf16-staged gru) -> p2
     with t on partitions: p2ps [125, 64].
  2) ratio chain: softplus via Exp/Ln (single act table set), clip [1,20].
  3) interp as ONE matmul per sample: lhsT Y [34,125] built via
     transpose -> line -> overlapped-reshape -> transpose; rhs M34 [34,2560].
  4) gain per compressor via min-1 form:
       rec' = thr/env  (recip_approx_fast of Act-Abs-scaled env)
       gain = 1 - clip((1-rec')*(1-rinv_k), 0, 0.9)
     computed in bf16 TT/TS ops (DVE 2x/4x modes), scan in bf16 with f32
     decay row and 128-col halo warmup + exact stream-start fixup.
  5) v = enh*s1 + (0.1*res)*s2 (10x scale folded into normalization),
     global abs-max via partition_all_reduce + AllGather(8).
"""

import numpy as np

import concourse.bass as bass
import concourse.bacc as bacc
import concourse.mybir as mybir
import concourse.tile as tile
from concourse.bass_utils import run_bass_kernel_spmd
from concourse import bass_isa

F32 = mybir.dt.float32
F32R = mybir.dt.float32r
BF16 = mybir.dt.bfloat16
I16 = mybir.dt.int16
AF = mybir.ActivationFunctionType
OP = mybir.AluOpType

NCORES = 8
S = 2
T = 4000
TB = S * T            # 8000
NSAMP = 320000
HOP = 80
GRU_H, H1, H2 = 256, 128, 64

P = 125               # audio partitions
CH = 2560             # cols per partition
HH = CH // 2
W = 128               # scan halo warmup cols
A = W + CH            # 2688

CHK = 500
NCHK = TB // CHK      # 16
TT = 125
LAM = 2.0 ** -10

_compiled = {}


def _prep_weights(W1, b1, a1, W2, b2, a2, W3, b3):
    W1 = W1.astype(np.float64); W2 = W2.astype(np.float64)
    w3 = W3.astype(np.float64)[2]
    b1 = b1.astype(np.float64); b2 = b2.astype(np.float64)
    b3r = float(np.asarray(b3, np.float64)[2])
    a1 = float(a1); a2 = float(a2)
    c1, d1 = (1 + a1) / 2, (1 - a1) / 2
    c2, d2 = (1 + a2) / 2, (1 - a2) / 2

    A2 = c1 * (W2 @ W1)
    B2 = d1 * W2
    beta2 = b2 + c1 * (W2 @ b1)

    a3 = c2 * (A2.T @ w3)
    b3v = c2 * (B2.T @ w3)
    c3v = d2 * w3
    gamma = c2 * float(w3 @ beta2) + b3r

    A2x = np.concatenate([A2, LAM * a3[None, :]], 0)
    B2x = np.concatenate([B2, LAM * b3v[None, :]], 0)
    beta2x = np.concatenate([beta2, [1.0]])
    r3 = np.concatenate([c3v, [1.0 / LAM]])
    spb = gamma - 1.0 / LAM

    W1T = W1.T
    out = {
        "w1t0": W1T[:128], "w1t1": W1T[128:],
        "a2xt0": A2x.T[:128], "a2xt1": A2x.T[128:],
        "b2xt": B2x.T,
        "r3": r3[:, None],
        "bias1": b1[:, None],
        "bias2": beta2x[:, None],
        "spbias": np.full((P, 1), spb),
    }
    return {k: np.ascontiguousarray(v, np.float32) for k, v in out.items()}


def _interp_m3():
    """[3, 80]: ratio_i[80t+k] = sum_j M3[j,k] * ratio[t-1+j]."""
    m = np.zeros((3, HOP), np.float64)
    for k in range(HOP):
        f = (k + 0.5) / HOP - 0.5
        if k < HOP // 2:
            m[0, k] = -f
            m[1, k] = 1.0 + f
        else:
            m[1, k] = 1.0 - f
            m[2, k] = f
    return np.ascontiguousarray(m, np.float32)


def _build_nc(sim=False, dbg=False):
    nc = bacc.Bacc("TRN2", target_bir_lowering=False, debug=False,
                   num_devices=NCORES)
    grut = nc.dram_tensor("grut", [GRU_H, TB], F32, kind="ExternalInput")
    enh = nc.dram_tensor("enh", [S, NSAMP], F32, kind="ExternalInput")
    noisy = nc.dram_tensor("noisy", [S, NSAMP], F32, kind="ExternalInput")
    wnames = ["w1t0", "w1t1", "a2xt0", "a2xt1", "b2xt", "r3",
              "bias1", "bias2", "spbias", "m3d", "ident"]
    wshapes = {"w1t0": [128, 128], "w1t1": [128, 128],
               "a2xt0": [128, 65], "a2xt1": [128, 65], "b2xt": [128, 65],
               "r3": [65, 1], "bias1": [128, 1], "bias2": [65, 1],
               "spbias": [P, 1], "m3d": [35, HOP], "ident": [128, 128]}
    wdram = {n: nc.dram_tensor(n, wshapes[n], F32, kind="ExternalInput")
             for n in wnames}
    out = nc.dram_tensor("out", [S, NSAMP], F32, kind="ExternalOutput")
    dbgd = {}
    if dbg:
        for dn, dshape in [("d_rat", [P, 32]), ("d_rinv", [P, CH]),
                           ("d_m1e", [P, CH]), ("d_m1r", [P, CH]),
                           ("d_ghe", [P, A]), ("d_sve", [P, A]),
                           ("d_ghr", [P, A]), ("d_svr", [P, A]),
                           ("d_v", [P, CH]), ("d_m2e", [P, CH]),
                           ("d_enh", [P, CH]), ("d_res", [P, CH])]:
            dbgd[dn] = nc.dram_tensor(dn, dshape, F32, kind="ExternalOutput")
    cc_in = nc.dram_tensor("cc_in", [2], F32)
    cc_out = nc.dram_tensor("cc_out", [2 * NCORES], F32, addr_space="Shared")

    with tile.TileContext(nc) as tc:
        with (
            tc.tile_pool(name="wpool", bufs=1) as wpool,
            tc.tile_pool(name="mlp", bufs=2) as mlp,
            tc.tile_pool(name="small", bufs=1) as small,
            tc.tile_pool(name="scr", bufs=1) as scr,
            tc.tile_pool(name="ps", bufs=2, space="PSUM") as ps,
            tc.tile_pool(name="ps1", bufs=1, space="PSUM") as ps1,
            tc.tile_pool(name="psi", bufs=1, space="PSUM") as psi,
            tc.tile_pool(name="psi2", bufs=2, space="PSUM") as psi2,
        ):
            # ---- resident weights ----
            wsb = {}
            for n in wnames:
                t_ = wpool.tile(wshapes[n], F32, tag=n, name=f"w_{n}")
                nc.sync.dma_start(t_[:], wdram[n][:])
                wsb[n] = t_
            wsr = {}
            for n in ("w1t0", "w1t1", "a2xt0", "a2xt1", "b2xt"):
                t_ = wpool.tile(wshapes[n], BF16, tag=n + "r", name=f"wr_{n}")
                nc.vector.tensor_copy(t_[:], wsb[n][:])
                wsr[n] = t_
            sh3 = wpool.tile([35, T + 34], BF16, tag="sh3")
            m3db = wpool.tile([35, HOP], BF16, tag="m3db")
            nc.vector.tensor_copy(m3db[:], wsb["m3d"][:])

            # decay row for scans; [0, W] = 0 is the stream-start fixup
            d0a = wpool.tile([P, A], F32, tag="d0a")
            nc.gpsimd.memset(d0a[:], 0.9)
            nc.gpsimd.memset(d0a[0:1, W:W + 1], 0.0)

            # per-partition act scale/bias constants
            scl = wpool.tile([128, 2], F32, tag="scl")
            nc.gpsimd.memset(scl[:, 0:1], 1.0 / 0.3)    # enh: 1/thr
            nc.gpsimd.memset(scl[:, 1:2], 100.0)        # res': 1/0.01
            epsb = wpool.tile([128, 2], F32, tag="epsb")
            nc.gpsimd.memset(epsb[:, 0:1], 1e-8 / 0.3)
            nc.gpsimd.memset(epsb[:, 1:2], 1e-7)

            p2ps = [ps1.tile([P, 32], F32, tag="p2", name=f"p2_{i}")
                    for i in range(S)]

            def pe_touch(ap):
                pass

            vemax = small.tile([P, 4 * S], F32, tag="vemax")

            # ================= gru staging (host-transposed, bf16) ========
            xt0 = wpool.tile([128, TB], BF16, tag="xt0")
            xt1 = wpool.tile([128, TB], BF16, tag="xt1")
            def gruq(q):
                qs = slice(q * (TB // 4), (q + 1) * (TB // 4))
                nc.gpsimd.dma_start(xt0[:, qs], grut[0:128, qs])
                nc.gpsimd.dma_start(xt1[:, qs], grut[128:256, qs])
            gruq(0)

            # ================= audio pre-ratio (no MLP dependency) ========
            audio = []   # per sample: dict of tiles
            for s in range(S):
                enh_t = scr.tile([P, CH], BF16, tag=f"enh{s}", name=f"enh{s}")
                res_t = scr.tile([P, CH], BF16, tag=f"res{s}", name=f"res{s}")
                nc.gpsimd.dma_start(enh_t[:],
                                    enh[s].rearrange("(p n) -> p n", p=P))
                nc.gpsimd.dma_start(res_t[:],
                                    noisy[s].rearrange("(p n) -> p n", p=P))
                if s == 0:
                    gruq(1)
                else:
                    gruq(2)
                    gruq(3)
                # res' = 0.1 * (noisy - enh)   (thr scales to 0.01)
                nc.vector.tensor_tensor(res_t[:], res_t[:], enh_t[:],
                                        op=OP.subtract)
                nc.vector.tensor_scalar(res_t[:], res_t[:], 0.1, None,
                                        op0=OP.mult)
                audio.append({"enh": enh_t, "res": res_t})

            # env -> rec -> m1 chains
            def env_chains(s):
                au = audio[s]
                for ci, (sig, sci) in enumerate(((au["enh"], 0),
                                                 (au["res"], 1))):
                    m1 = scr.tile([P, CH], BF16, tag=f"m1_{s}{ci}",
                                  name=f"m1_{s}{ci}")
                    # env' = |x/thr + eps'| (eps folded into bias; no bf16
                    # input can cancel it to 0 exactly)
                    envf = scr.tile([P, CH], F32, tag=f"envf{ci}",
                                    name=f"envf{ci}_{s}")
                    nc.scalar.activation(envf[:], sig[:], AF.Abs,
                                         scale=scl[:P, sci:sci + 1],
                                         bias=epsb[:P, sci:sci + 1])
                    if ci == 0:
                        # emax' = max(env') ; true emax = 0.3 * emax'
                        nc.vector.tensor_reduce(
                            vemax[:, 2 * S + 2 * s:2 * S + 2 * s + 1],
                            envf[:, 0:HH],
                            op=OP.max, axis=mybir.AxisListType.X)
                        nc.vector.tensor_reduce(
                            vemax[:, 2 * S + 2 * s + 1:2 * S + 2 * s + 2],
                            envf[:, HH:CH],
                            op=OP.max, axis=mybir.AxisListType.X)
                    recf = scr.tile([P, CH], F32, tag=f"recf{ci}",
                                    name=f"recf{ci}_{s}")
                    nc.vector.reciprocal_approx_fast(out=recf[:],
                                                     in_=envf[:])
                    # m1 = relu(1 - rec')   (bf16)
                    nc.scalar.activation(m1[:], recf[:], AF.Relu,
                                         bias=1.0, scale=-1.0)
                    au[f"m1_{ci}"] = m1

            env_chains(0)
            env_chains(1)

            # ================= post-ratio audio ===========================
            def sample_block(s):
                au = audio[s]
                # ---- ratio chain [125, 32] ----
                # softplus(x) = relu(x) + poly(e^-|x|); Exp/Abs/Relu all
                # live in act-table set 0 -> no table switches
                px = small.tile([P, 32], F32, tag=f"px{s}", name=f"px{s}")
                nc.vector.tensor_copy(px[:], p2ps[s][:])
                ax = small.tile([P, 32], F32, tag=f"ax{s}", name=f"ax{s}")
                nc.scalar.activation(ax[:], px[:], AF.Abs,
                                     bias=wsb["spbias"][:])
                uu = small.tile([P, 32], F32, tag=f"uu{s}", name=f"uu{s}")
                nc.scalar.activation(uu[:], ax[:], AF.Exp, scale=-1.0)
                hh = small.tile([P, 32], F32, tag=f"hh{s}", name=f"hh{s}")
                nc.vector.tensor_scalar(hh[:], uu[:], 0.11477816,
                                        -0.40741059, op0=OP.mult, op1=OP.add)
                nc.vector.tensor_tensor(hh[:], hh[:], uu[:], op=OP.mult)
                nc.vector.tensor_scalar(hh[:], hh[:], 0.98669098, None,
                                        op0=OP.add)
                nc.vector.tensor_tensor(hh[:], hh[:], uu[:], op=OP.mult)
                rat = small.tile([P, 32], F32, tag=f"rat{s}", name=f"rat{s}")
                nc.vector.tensor_scalar(rat[:], px[:],
                                        wsb["spbias"][:P, 0:1], 0.0,
                                        op0=OP.add, op1=OP.max)
                nc.vector.tensor_tensor(rat[:], rat[:], hh[:], op=OP.add)
                nc.vector.tensor_scalar(rat[:], rat[:], 1.0, 20.0,
                                        op0=OP.add, op1=OP.min)
                if dbg and s == 0:
                    nc.sync.dma_start(dbgd["d_rat"][:], rat[:])
                # ---- sh3 rows (baseline scheme): row b+j col i =
                # ratio_s(i + j - 1), clipped at stream edges ----
                ratT_ps = psi.tile([32, P], F32, tag="rT")
                pe_touch(rat)
                nc.tensor.transpose(ratT_ps[:], rat[:], wsb["ident"][:P, :P])
                ratT = small.tile([32, P], BF16, tag=f"ratT{s}",
                                  name=f"ratT{s}")
                nc.scalar.copy(ratT[:], ratT_ps[:])
                b = 32 * s
                rT = ratT[:]
                r3d = lambda ap: ap.rearrange("p (r q) -> p r q", q=P)
                nc.sync.dma_start(r3d(sh3[b:b + 1, 1:T + 1]), rT)
                nc.sync.dma_start(sh3[b:b + 1, 0:1], rT[0:1, 0:1])
                nc.sync.dma_start(r3d(sh3[b + 1:b + 2, 0:T]), rT)
                nc.sync.dma_start(sh3[b + 2:b + 3, 0:124], rT[0:1, 1:P])
                nc.sync.dma_start(
                    r3d(sh3[b + 2:b + 3, 124:124 + 31 * P]), rT[1:32, :])
                nc.sync.dma_start(sh3[b + 2:b + 3, T - 1:T],
                                  rT[31:32, P - 1:P])
                # ---- interp (group matmuls) + rinv from psum ----
                rinv = scr.tile([P, CH], F32, tag="rcr", name=f"rinv{s}")
                pe_touch(sh3[0:1, 0:1] if s == 0 else sh3[0:1, 1:2])
                for g in range(6):
                    taus = list(range(g * 6, min((g + 1) * 6, 32)))
                    rips = psi2.tile([P, 480], F32, tag="rips")
                    for ti, tau in enumerate(taus):
                        lhsT = sh3[b:b + 3, tau:tau + 32 * P:32]
                        nc.tensor.matmul(rips[:, ti * HOP:(ti + 1) * HOP],
                                         lhsT, m3db[b:b + 3, :],
                                         start=True, stop=True)
                    nwid = len(taus) * HOP
                    nc.vector.reciprocal_approx_fast(
                        out=rinv[:, g * 480:g * 480 + nwid],
                        in_=rips[:, :nwid])
                if dbg and s == 0:
                    nc.sync.dma_start(dbgd["d_rinv"][:], rinv[:])
                m2e = scr.tile([P, CH], BF16, tag=f"m2e{s}", name=f"m2e{s}")
                nc.scalar.activation(m2e[:], rinv[:], AF.Copy,
                                     bias=1.0, scale=-1.0)
                m2r = scr.tile([P, CH], BF16, tag=f"m2r{s}", name=f"m2r{s}")
                nc.scalar.activation(m2r[:], m2e[:], AF.Copy,
                                     bias=-1.0, scale=2.0)

                if dbg and s == 0:
                    nc.gpsimd.dma_start(dbgd["d_m2e"][:], m2e[:])
                    nc.gpsimd.dma_start(dbgd["d_m1e"][:], au["m1_0"][:])
                    nc.gpsimd.dma_start(dbgd["d_m1r"][:], au["m1_1"][:])
                    nc.gpsimd.dma_start(dbgd["d_enh"][:], au["enh"][:])
                    nc.gpsimd.dma_start(dbgd["d_res"][:], au["res"][:])
                # ---- gains + scans + combine ----
                svs = []
                for ci, m2 in enumerate((m2e, m2r)):
                    m1 = au[f"m1_{ci}"]
                    gh = scr.tile([P, A], BF16, tag=f"gh{ci}",
                                  name=f"gh{s}{ci}")
                    gm = gh[:, W:A]
                    nc.vector.tensor_tensor(gm, m1[:], m2[:], op=OP.mult)
                    nc.vector.tensor_scalar(gm, gm, -1.0, 0.9,
                                            op0=OP.max, op1=OP.min)
                    nc.scalar.activation(gm, gm, AF.Copy,
                                         bias=1.0, scale=-1.0)
                    # halo: prev partition's last W gains
                    ght = scr.tile([P, W], BF16, tag=f"ght{ci}",
                                   name=f"ght{s}{ci}")
                    nc.vector.tensor_copy(ght[:], gh[:, A - W:A])
                    nc.sync.dma_start(gh[1:P, 0:W], ght[0:P - 1, :])
                    nc.gpsimd.memset(gh[0:1, 0:W], 1.0)
                    # stream-start fixup: s[0, W] must be 10*g0
                    nc.vector.tensor_scalar(gh[0:1, W:W + 1],
                                            gh[0:1, W:W + 1], 10.0, None,
                                            op0=OP.mult)
                    sv = scr.tile([P, A], BF16, tag=f"sv{ci}",
                                  name=f"sv{s}{ci}")
                    nc.vector.tensor_tensor_scan(sv[:], d0a[:], gh[:], 0.0,
                                                 op0=OP.mult, op1=OP.add)
                    if dbg and s == 0:
                        nm = "e" if ci == 0 else "r"
                        nc.gpsimd.dma_start(dbgd["d_gh" + nm][:], gh[:])
                        nc.gpsimd.dma_start(dbgd["d_sv" + nm][:], sv[:])
                    svs.append(sv)

                # v = enh*s1 + res'*s2 (both scans carry 10x; folded later)
                ce = scr.tile([P, CH], BF16, tag=f"ce{s}", name=f"ce{s}")
                nc.vector.tensor_tensor(ce[:], au["enh"][:], svs[0][:, W:A],
                                        op=OP.mult)
                cr = scr.tile([P, CH], BF16, tag="rcr", name=f"cr{s}")
                nc.vector.tensor_tensor(cr[:], au["res"][:], svs[1][:, W:A],
                                        op=OP.mult)
                nc.vector.tensor_tensor(ce[:], ce[:], cr[:], op=OP.add)
                for h in range(2):
                    hs = slice(HH * h, HH * (h + 1))
                    nc.vector.tensor_reduce(
                        vemax[:, 2 * s + h:2 * s + h + 1], ce[:, hs],
                        op=OP.max, axis=mybir.AxisListType.X,
                        apply_absolute_value=True)
                if dbg and s == 0:
                    nc.gpsimd.dma_start(dbgd["d_v"][:], ce[:])
                au["v"] = ce

            # ================= MLP chunks =================================
            for c in range(NCHK):
                cs = slice(c * CHK, (c + 1) * CHK)
                x0 = xt0[:, cs]
                x1 = xt1[:, cs]

                yps = ps.tile([128, CHK], F32, tag="yz")
                nc.tensor.matmul(yps[:], wsr["w1t0"][:], x0,
                                 start=True, stop=False)
                nc.tensor.matmul(yps[:], wsr["w1t1"][:], x1,
                                 start=False, stop=True)
                ay = mlp.tile([128, CHK], BF16, tag="ay")
                nc.scalar.activation(ay[:], yps[:], AF.Abs,
                                     bias=wsb["bias1"][:])

                zfull = ps.tile([128, CHK], F32, tag="zz")
                zps = zfull[0:65, :]
                nc.tensor.matmul(zps[:], wsr["a2xt0"][:], x0,
                                 start=True, stop=False)
                nc.tensor.matmul(zps[:], wsr["a2xt1"][:], x1,
                                 start=False, stop=False)
                nc.tensor.matmul(zps[:], wsr["b2xt"][:], ay[:],
                                 start=False, stop=True)
                t2 = mlp.tile([65, CHK], F32, tag="t2")
                nc.scalar.activation(t2[:], zps[:], AF.Abs,
                                     bias=wsb["bias2"][:])

                for j in range(CHK // TT):
                    cc = c * (CHK // TT) + j
                    nc.tensor.matmul(p2ps[cc // 32][:, cc % 32:cc % 32 + 1],
                                     t2[:, j * TT:(j + 1) * TT],
                                     wsb["r3"][:], start=True, stop=True)
                if c == NCHK // 2 - 1:
                    sample_block(0)
                elif c == NCHK - 1:
                    sample_block(1)

            # ================= global normalization =======================
            galr = small.tile([P, 4 * S], F32, tag="galr")
            nc.gpsimd.partition_all_reduce(galr[:], vemax[:], channels=P,
                                           reduce_op=bass_isa.ReduceOp.max)
            gmr = small.tile([P, 2], F32, tag="gmr")
            nc.vector.tensor_reduce(gmr[0:1, 0:1], galr[0:1, 0:2 * S],
                                    op=OP.max, axis=mybir.AxisListType.X)
            nc.vector.tensor_reduce(gmr[0:1, 1:2],
                                    galr[0:1, 2 * S:4 * S],
                                    op=OP.max, axis=mybir.AxisListType.X)
            ccsb = small.tile([1, 2 * NCORES], F32, tag="ccsb")
            if sim:
                gbc = small.tile([NCORES, 2], F32, tag="gbc")
                nc.gpsimd.partition_broadcast(gbc[:], gmr[0:1, 0:2],
                                              channels=NCORES)
                nc.sync.dma_start(ccsb[0:1, 0:NCORES],
                                  gbc[0:NCORES // 2, :].rearrange(
                                      "p r -> (p r)")[None, :])
                nc.sync.dma_start(ccsb[0:1, NCORES:2 * NCORES],
                                  gbc[NCORES // 2:NCORES, :].rearrange(
                                      "p r -> (p r)")[None, :])
            else:
                with tc.tile_critical():
                    cc_sem = nc.alloc_semaphore("ccs")
                    nc.gpsimd.dma_start(cc_in[:], gmr[0:1, 0:2]).then_inc(
                        cc_sem, 16)
                    nc.gpsimd.collective_compute(
                        "AllGather", OP.bypass,
                        replica_groups=[list(range(NCORES))],
                        ins=[cc_in[:]], outs=[cc_out[:]],
                    )._wait_ge(cc_sem, 16).then_inc(cc_sem, 1)
                    nc.gpsimd.dma_start(ccsb[:], cc_out[None, :])._wait_ge(
                        cc_sem, 17).then_inc(cc_sem, 16)
                    nc.gpsimd.engine_nop()._wait_ge(cc_sem, 33)

            sg = small.tile([1, 4], F32, tag="sg")
            nc.vector.tensor_reduce(sg[:, 0:1], ccsb[:, 0:2 * NCORES:2],
                                    op=OP.max, axis=mybir.AxisListType.X)
            nc.vector.tensor_reduce(sg[:, 1:2], ccsb[:, 1:2 * NCORES:2],
                                    op=OP.max, axis=mybir.AxisListType.X)
            # sigma = 0.3*emax' / (vmax + 1e-7)  (v is 10x; 0.1 folded)
            nc.vector.tensor_scalar(sg[:, 2:3], sg[:, 0:1], 1e-7, None,
                                    op0=OP.add)
            nc.vector.reciprocal_approx_fast(out=sg[:, 0:1], in_=sg[:, 2:3])
            nc.vector.tensor_scalar(sg[:, 1:2], sg[:, 1:2], 0.3, None,
                                    op0=OP.mult)
            nc.vector.tensor_tensor(sg[:, 3:4], sg[:, 0:1], sg[:, 1:2],
                                    op=OP.mult)
            sgb = small.tile([P, 1], F32, tag="sgb")
            nc.gpsimd.partition_broadcast(sgb[:], sg[0:1, 3:4], channels=P)

            for s in range(S):
                v = audio[s]["v"]
                vf = scr.tile([P, CH], F32, tag=f"envf{s}", name=f"vf{s}")
                ov = out[s].rearrange("(p n) -> p n", p=P)
                for h in range(2):
                    hs = slice(HH * h, HH * (h + 1))
                    nc.vector.tensor_scalar(vf[:, hs], v[:, hs],
                                            sgb[:, 0:1], None, op0=OP.mult)
                    nc.sync.dma_start(ov[:, hs], vf[:, hs])
    nc.finalize()
    return nc


def kernel(trace=False, **inputs):
    gru = np.ascontiguousarray(np.asarray(inputs["gru_output"], np.float32))
    enh = np.ascontiguousarray(np.asarray(inputs["enhanced"], np.float32))
    noisy = np.ascontiguousarray(np.asarray(inputs["noisy"], np.float32))
    B = gru.shape[0]
    wts = _prep_weights(inputs["W1"], inputs["b1"], inputs["a1"],
                        inputs["W2"], inputs["b2"], inputs["a2"],
                        inputs["W3"], inputs["b3"])
    m3 = _interp_m3()
    m3d = np.zeros((35, HOP), np.float32)
    m3d[0:3] = m3
    m3d[32:35] = m3
    wts["m3d"] = m3d
    wts["ident"] = np.ascontiguousarray(np.eye(128, dtype=np.float32))

    if "nc" not in _compiled:
        _compiled["nc"] = _build_nc()
    nc = _compiled["nc"]

    per = B // NCORES
    in_maps = []
    for c in range(NCORES):
        m = {
            "grut": np.ascontiguousarray(
                gru[c * per:(c + 1) * per].reshape(TB, GRU_H).T),
            "enh": np.ascontiguousarray(enh[c * per:(c + 1) * per]),
            "noisy": np.ascontiguousarray(noisy[c * per:(c + 1) * per]),
        }
        m.update(wts)
        in_maps.append(m)

    res = run_bass_kernel_spmd(nc, in_maps, list(range(NCORES)), trace=trace)
    outs = [res.results[c]["out"] for c in range(NCORES)]
    full = np.concatenate(outs, axis=0)
    if trace:
        return full, res
    return full


if __name__ == "__main__":
    pass


# revision 5
# speedup vs baseline: 1.0233x; 1.0049x over previous
"""NeuralWDRC Trainium2 kernel v2: 8-core data-parallel (2 samples/core).

Per core:
  1) MLP (baseline |x|-folded form, f32r matmuls, bf16-staged gru) -> p2
     with t on partitions: p2ps [125, 64].
  2) ratio chain: softplus via Exp/Ln (single act table set), clip [1,20].
  3) interp as ONE matmul per sample: lhsT Y [34,125] built via
     transpose -> line -> overlapped-reshape -> transpose; rhs M34 [34,2560].
  4) gain per compressor via min-1 form:
       rec' = thr/env  (recip_approx_fast of Act-Abs-scaled env)
       gain = 1 - clip((1-rec')*(1-rinv_k), 0, 0.9)
     computed in bf16 TT/TS ops (DVE 2x/4x modes), scan in bf16 with f32
     decay row and 128-col halo warmup + exact stream-start fixup.
  5) v = enh*s1 + (0.1*res)*s2 (10x scale folded into normalization),
     global abs-max via partition_all_reduce + AllGather(8).
"""

import numpy as np

import concourse.bass as bass
import concourse.bacc as bacc
import concourse.mybir as mybir
import concourse.tile as tile
from concourse.bass_utils import run_bass_kernel_spmd
from concourse import bass_isa

F32 = mybir.dt.float32
F32R = mybir.dt.float32r
BF16 = mybir.dt.bfloat16
I16 = mybir.dt.int16
AF = mybir.ActivationFunctionType
OP = mybir.AluOpType

NCORES = 8
S = 2
T = 4000
TB = S * T            # 8000
NSAMP = 320000
HOP = 80
GRU_H, H1, H2 = 256, 128, 64

P = 125               # audio partitions
CH = 2560             # cols per partition
HH = CH // 2
W = 128               # scan halo warmup cols
A = W + CH            # 2688

CHK = 500
NCHK = TB // CHK      # 16
TT = 125
LAM = 2.0 ** -10

_compiled = {}


def _prep_weights(W1, b1, a1, W2, b2, a2, W3, b3):
    W1 = W1.astype(np.float64); W2 = W2.astype(np.float64)
    w3 = W3.astype(np.float64)[2]
    b1 = b1.astype(np.float64); b2 = b2.astype(np.float64)
    b3r = float(np.asarray(b3, np.float64)[2])
    a1 = float(a1); a2 = float(a2)
    c1, d1 = (1 + a1) / 2, (1 - a1) / 2
    c2, d2 = (1 + a2) / 2, (1 - a2) / 2

    A2 = c1 * (W2 @ W1)
    B2 = d1 * W2
    beta2 = b2 + c1 * (W2 @ b1)

    a3 = c2 * (A2.T @ w3)
    b3v = c2 * (B2.T @ w3)
    c3v = d2 * w3
    gamma = c2 * float(w3 @ beta2) + b3r

    A2x = np.concatenate([A2, LAM * a3[None, :]], 0)
    B2x = np.concatenate([B2, LAM * b3v[None, :]], 0)
    beta2x = np.concatenate([beta2, [1.0]])
    r3 = np.concatenate([c3v, [1.0 / LAM]])
    spb = gamma - 1.0 / LAM

    W1T = W1.T
    out = {
        "w1t0": W1T[:128], "w1t1": W1T[128:],
        "a2xt0": A2x.T[:128], "a2xt1": A2x.T[128:],
        "b2xt": B2x.T,
        "r3": r3[:, None],
        "bias1": b1[:, None],
        "bias2": beta2x[:, None],
        "spbias": np.full((P, 1), spb),
    }
    return {k: np.ascontiguousarray(v, np.float32) for k, v in out.items()}


def _interp_m3():
    """[3, 80]: ratio_i[80t+k] = sum_j M3[j,k] * ratio[t-1+j]."""
    m = np.zeros((3, HOP), np.float64)
    for k in range(HOP):
        f = (k + 0.5) / HOP - 0.5
        if k < HOP // 2:
            m[0, k] = -f
            m[1, k] = 1.0 + f
        else:
            m[1, k] = 1.0 - f
            m[2, k] = f
    return np.ascontiguousarray(m, np.float32)


def _build_nc(sim=False, dbg=False):
    nc = bacc.Bacc("TRN2", target_bir_lowering=False, debug=False,
                   num_devices=NCORES)
    grut = nc.dram_tensor("grut", [GRU_H, TB], F32, kind="ExternalInput")
    enh = nc.dram_tensor("enh", [S, NSAMP], F32, kind="ExternalInput")
    noisy = nc.dram_tensor("noisy", [S, NSAMP], F32, kind="ExternalInput")
    wnames = ["w1t0", "w1t1", "a2xt0", "a2xt1", "b2xt", "r3",
              "bias1", "bias2", "spbias", "m3d", "ident"]
    wshapes = {"w1t0": [128, 128], "w1t1": [128, 128],
               "a2xt0": [128, 65], "a2xt1": [128, 65], "b2xt": [128, 65],
               "r3": [65, 1], "bias1": [128, 1], "bias2": [65, 1],
               "spbias": [P, 1], "m3d": [35, HOP], "ident": [128, 128]}
    wdram = {n: nc.dram_tensor(n, wshapes[n], F32, kind="ExternalInput")
             for n in wnames}
    out = nc.dram_tensor("out", [S, NSAMP], F32, kind="ExternalOutput")
    dbgd = {}
    if dbg:
        for dn, dshape in [("d_rat", [P, 32]), ("d_rinv", [P, CH]),
                           ("d_m1e", [P, CH]), ("d_m1r", [P, CH]),
                           ("d_ghe", [P, A]), ("d_sve", [P, A]),
                           ("d_ghr", [P, A]), ("d_svr", [P, A]),
                           ("d_v", [P, CH]), ("d_m2e", [P, CH]),
                           ("d_enh", [P, CH]), ("d_res", [P, CH])]:
            dbgd[dn] = nc.dram_tensor(dn, dshape, F32, kind="ExternalOutput")
    cc_in = nc.dram_tensor("cc_in", [2], F32)
    cc_out = nc.dram_tensor("cc_out", [2 * NCORES], F32, addr_space="Shared")

    with tile.TileContext(nc) as tc:
        with (
            tc.tile_pool(name="wpool", bufs=1) as wpool,
            tc.tile_pool(name="mlp", bufs=2) as mlp,
            tc.tile_pool(name="small", bufs=1) as small,
            tc.tile_pool(name="scr", bufs=1) as scr,
            tc.tile_pool(name="ps", bufs=2, space="PSUM") as ps,
            tc.tile_pool(name="ps1", bufs=1, space="PSUM") as ps1,
            tc.tile_pool(name="psi", bufs=1, space="PSUM") as psi,
            tc.tile_pool(name="psi2", bufs=2, space="PSUM") as psi2,
        ):
            # ---- resident weights ----
            wsb = {}
            for n in wnames:
                t_ = wpool.tile(wshapes[n], F32, tag=n, name=f"w_{n}")
                nc.sync.dma_start(t_[:], wdram[n][:])
                wsb[n] = t_
            wsr = {}
            for n in ("w1t0", "w1t1", "a2xt0", "a2xt1", "b2xt"):
                t_ = wpool.tile(wshapes[n], BF16, tag=n + "r", name=f"wr_{n}")
                nc.vector.tensor_copy(t_[:], wsb[n][:])
                wsr[n] = t_
            sh3 = wpool.tile([35, T + 34], BF16, tag="sh3")
            m3db = wpool.tile([35, HOP], BF16, tag="m3db")
            nc.vector.tensor_copy(m3db[:], wsb["m3d"][:])

            # decay row for scans; [0, W] = 0 is the stream-start fixup
            d0a = wpool.tile([P, A], F32, tag="d0a")
            nc.gpsimd.memset(d0a[:], 0.9)
            nc.gpsimd.memset(d0a[0:1, W:W + 1], 0.0)

            # per-partition act scale/bias constants
            scl = wpool.tile([128, 2], F32, tag="scl")
            nc.gpsimd.memset(scl[:, 0:1], 1.0 / 0.3)    # enh: 1/thr
            nc.gpsimd.memset(scl[:, 1:2], 100.0)        # res': 1/0.01
            epsb = wpool.tile([128, 2], F32, tag="epsb")
            nc.gpsimd.memset(epsb[:, 0:1], 1e-8 / 0.3)
            nc.gpsimd.memset(epsb[:, 1:2], 1e-7)

            p2ps = [ps1.tile([P, 32], F32, tag="p2", name=f"p2_{i}")
                    for i in range(S)]

            def pe_touch(ap):
                pass

            vemax = small.tile([P, 4 * S], F32, tag="vemax")

            # ================= gru staging (host-transposed, bf16) ========
            xt0 = wpool.tile([128, TB], BF16, tag="xt0")
            xt1 = wpool.tile([128, TB], BF16, tag="xt1")
            def gruq(q):
                qs = slice(q * (TB // 4), (q + 1) * (TB // 4))
                nc.gpsimd.dma_start(xt0[:, qs], grut[0:128, qs])
                nc.gpsimd.dma_start(xt1[:, qs], grut[128:256, qs])
            gruq(0)

            # ================= audio pre-ratio (no MLP dependency) ========
            audio = []   # per sample: dict of tiles
            for s in range(S):
                enh_t = scr.tile([P, CH], BF16, tag=f"enh{s}", name=f"enh{s}")
                res_t = scr.tile([P, CH], BF16, tag=f"res{s}", name=f"res{s}")
                nc.gpsimd.dma_start(enh_t[:],
                                    enh[s].rearrange("(p n) -> p n", p=P))
                nc.gpsimd.dma_start(res_t[:],
                                    noisy[s].rearrange("(p n) -> p n", p=P))
                if s == 0:
                    gruq(1)
                else:
                    gruq(2)
                    gruq(3)
                # res' = 0.1 * (noisy - enh)   (thr scales to 0.01)
                nc.vector.tensor_tensor(res_t[:], res_t[:], enh_t[:],
                                        op=OP.subtract)
                nc.vector.tensor_scalar(res_t[:], res_t[:], 0.1, None,
                                        op0=OP.mult)
                audio.append({"enh": enh_t, "res": res_t})

            # env -> rec -> m1 chains
            def env_chains(s):
                au = audio[s]
                for ci, (sig, sci) in enumerate(((au["enh"], 0),
                                                 (au["res"], 1))):
                    m1 = scr.tile([P, CH], BF16, tag=f"m1_{s}{ci}",
                                  name=f"m1_{s}{ci}")
                    # env' = |x/thr + eps'| (eps folded into bias; no bf16
                    # input can cancel it to 0 exactly)
                    envf = scr.tile([P, CH], F32, tag=f"envf{ci}",
                                    name=f"envf{ci}_{s}")
                    nc.scalar.activation(envf[:], sig[:], AF.Abs,
                                         scale=scl[:P, sci:sci + 1],
                                         bias=epsb[:P, sci:sci + 1])
                    if ci == 0:
                        # emax' = max(env') ; true emax = 0.3 * emax'
                        nc.vector.tensor_reduce(
                            vemax[:, 2 * S + 2 * s:2 * S + 2 * s + 1],
                            envf[:, 0:HH],
                            op=OP.max, axis=mybir.AxisListType.X)
                        nc.vector.tensor_reduce(
                            vemax[:, 2 * S + 2 * s + 1:2 * S + 2 * s + 2],
                            envf[:, HH:CH],
                            op=OP.max, axis=mybir.AxisListType.X)
                    recf = scr.tile([P, CH], F32, tag=f"recf{ci}",
                                    name=f"recf{ci}_{s}")
                    nc.vector.reciprocal_approx_fast(out=recf[:],
                                                     in_=envf[:])
                    # m1 = relu(1 - rec')   (bf16)
                    nc.scalar.activation(m1[:], recf[:], AF.Relu,
                                         bias=1.0, scale=-1.0)
                    au[f"m1_{ci}"] = m1

            env_chains(0)
            env_chains(1)

            # ================= post-ratio audio ===========================
            def sample_block(s):
                au = audio[s]
                # ---- ratio chain [125, 32] ----
                # softplus(x) = relu(x) + poly(e^-|x|); Exp/Abs/Relu all
                # live in act-table set 0 -> no table switches
                px = small.tile([P, 32], F32, tag=f"px{s}", name=f"px{s}")
                nc.vector.tensor_copy(px[:], p2ps[s][:])
                ax = small.tile([P, 32], F32, tag=f"ax{s}", name=f"ax{s}")
                nc.scalar.activation(ax[:], px[:], AF.Abs,
                                     bias=wsb["spbias"][:])
                uu = small.tile([P, 32], F32, tag=f"uu{s}", name=f"uu{s}")
                nc.scalar.activation(uu[:], ax[:], AF.Exp, scale=-1.0)
                hh = small.tile([P, 32], F32, tag=f"hh{s}", name=f"hh{s}")
                nc.vector.tensor_scalar(hh[:], uu[:], 0.11477816,
                                        -0.40741059, op0=OP.mult, op1=OP.add)
                nc.vector.tensor_tensor(hh[:], hh[:], uu[:], op=OP.mult)
                nc.vector.tensor_scalar(hh[:], hh[:], 0.98669098, None,
                                        op0=OP.add)
                nc.vector.tensor_tensor(hh[:], hh[:], uu[:], op=OP.mult)
                rat = small.tile([P, 32], F32, tag=f"rat{s}", name=f"rat{s}")
                nc.vector.tensor_scalar(rat[:], px[:],
                                        wsb["spbias"][:P, 0:1], 0.0,
                                        op0=OP.add, op1=OP.max)
                nc.vector.tensor_tensor(rat[:], rat[:], hh[:], op=OP.add)
                nc.vector.tensor_scalar(rat[:], rat[:], 1.0, 20.0,
                                        op0=OP.add, op1=OP.min)
                if dbg and s == 0:
                    nc.sync.dma_start(dbgd["d_rat"][:], rat[:])
                # ---- sh3 rows (baseline scheme): row b+j col i =
                # ratio_s(i + j - 1), clipped at stream edges ----
                ratT_ps = psi.tile([32, P], F32, tag="rT")
                pe_touch(rat)
                nc.tensor.transpose(ratT_ps[:], rat[:], wsb["ident"][:P, :P])
                ratT = small.tile([32, P], BF16, tag=f"ratT{s}",
                                  name=f"ratT{s}")
                nc.scalar.copy(ratT[:], ratT_ps[:])
                b = 32 * s
                rT = ratT[:]
                r3d = lambda ap: ap.rearrange("p (r q) -> p r q", q=P)
                nc.sync.dma_start(r3d(sh3[b:b + 1, 1:T + 1]), rT)
                nc.sync.dma_start(sh3[b:b + 1, 0:1], rT[0:1, 0:1])
                nc.sync.dma_start(r3d(sh3[b + 1:b + 2, 0:T]), rT)
                nc.sync.dma_start(sh3[b + 2:b + 3, 0:124], rT[0:1, 1:P])
                nc.sync.dma_start(
                    r3d(sh3[b + 2:b + 3, 124:124 + 31 * P]), rT[1:32, :])
                nc.sync.dma_start(sh3[b + 2:b + 3, T - 1:T],
                                  rT[31:32, P - 1:P])
                # ---- interp (group matmuls) + rinv from psum ----
                rinv = scr.tile([P, CH], F32, tag="rcr", name=f"rinv{s}")
                pe_touch(sh3[0:1, 0:1] if s == 0 else sh3[0:1, 1:2])
                for g in range(6):
                    taus = list(range(g * 6, min((g + 1) * 6, 32)))
                    rips = psi2.tile([P, 480], F32, tag="rips")
                    for ti, tau in enumerate(taus):
                        lhsT = sh3[b:b + 3, tau:tau + 32 * P:32]
                        nc.tensor.matmul(rips[:, ti * HOP:(ti + 1) * HOP],
                                         lhsT, m3db[b:b + 3, :],
                                         start=True, stop=True)
                    nwid = len(taus) * HOP
                    nc.vector.reciprocal_approx_fast(
                        out=rinv[:, g * 480:g * 480 + nwid],
                        in_=rips[:, :nwid])
                if dbg and s == 0:
                    nc.sync.dma_start(dbgd["d_rinv"][:], rinv[:])
                m2e = scr.tile([P, CH], BF16, tag=f"m2e{s}", name=f"m2e{s}")
                m2r = scr.tile([P, CH], BF16, tag=f"m2r{s}", name=f"m2r{s}")
                for h in range(2):
                    hs = slice(HH * h, HH * (h + 1))
                    nc.scalar.activation(m2e[:, hs], rinv[:, hs], AF.Copy,
                                         bias=1.0, scale=-1.0)
                    nc.scalar.activation(m2r[:, hs], m2e[:, hs], AF.Copy,
                                         bias=-1.0, scale=2.0)

                if dbg and s == 0:
                    nc.gpsimd.dma_start(dbgd["d_m2e"][:], m2e[:])
                    nc.gpsimd.dma_start(dbgd["d_m1e"][:], au["m1_0"][:])
                    nc.gpsimd.dma_start(dbgd["d_m1r"][:], au["m1_1"][:])
                    nc.gpsimd.dma_start(dbgd["d_enh"][:], au["enh"][:])
                    nc.gpsimd.dma_start(dbgd["d_res"][:], au["res"][:])
                # ---- gains + scans + combine ----
                svs = []
                for ci, m2 in enumerate((m2e, m2r)):
                    m1 = au[f"m1_{ci}"]
                    gh = scr.tile([P, A], BF16, tag=f"gh{ci}",
                                  name=f"gh{s}{ci}")
                    for h in range(2):
                        hs = slice(HH * h, HH * (h + 1))
                        gmh = gh[:, W + HH * h:W + HH * (h + 1)]
                        nc.vector.tensor_tensor(gmh, m1[:, hs], m2[:, hs],
                                                op=OP.mult)
                        nc.vector.tensor_scalar(gmh, gmh, -1.0, 0.9,
                                                op0=OP.max, op1=OP.min)
                        nc.scalar.activation(gmh, gmh, AF.Copy,
                                             bias=1.0, scale=-1.0)
                    # halo: prev partition's last W gains
                    ght = scr.tile([P, W], BF16, tag=f"ght{ci}",
                                   name=f"ght{s}{ci}")
                    nc.vector.tensor_copy(ght[:], gh[:, A - W:A])
                    nc.sync.dma_start(gh[1:P, 0:W], ght[0:P - 1, :])
                    nc.gpsimd.memset(gh[0:1, 0:W], 1.0)
                    # stream-start fixup: s[0, W] must be 10*g0
                    nc.vector.tensor_scalar(gh[0:1, W:W + 1],
                                            gh[0:1, W:W + 1], 10.0, None,
                                            op0=OP.mult)
                    sv = scr.tile([P, A], BF16, tag=f"sv{ci}",
                                  name=f"sv{s}{ci}")
                    nc.vector.tensor_tensor_scan(sv[:], d0a[:], gh[:], 0.0,
                                                 op0=OP.mult, op1=OP.add)
                    if dbg and s == 0:
                        nm = "e" if ci == 0 else "r"
                        nc.gpsimd.dma_start(dbgd["d_gh" + nm][:], gh[:])
                        nc.gpsimd.dma_start(dbgd["d_sv" + nm][:], sv[:])
                    svs.append(sv)

                # v = enh*s1 + res'*s2 (both scans carry 10x; folded later)
                ce = scr.tile([P, CH], BF16, tag=f"ce{s}", name=f"ce{s}")
                nc.vector.tensor_tensor(ce[:], au["enh"][:], svs[0][:, W:A],
                                        op=OP.mult)
                cr = scr.tile([P, CH], BF16, tag="rcr", name=f"cr{s}")
                nc.vector.tensor_tensor(cr[:], au["res"][:], svs[1][:, W:A],
                                        op=OP.mult)
                nc.vector.tensor_tensor(ce[:], ce[:], cr[:], op=OP.add)
                for h in range(2):
                    hs = slice(HH * h, HH * (h + 1))
                    nc.vector.tensor_reduce(
                        vemax[:, 2 * s + h:2 * s + h + 1], ce[:, hs],
                        op=OP.max, axis=mybir.AxisListType.X,
                        apply_absolute_value=True)
                if dbg and s == 0:
                    nc.gpsimd.dma_start(dbgd["d_v"][:], ce[:])
                au["v"] = ce

            # ================= MLP chunks =================================
            for c in range(NCHK):
                cs = slice(c * CHK, (c + 1) * CHK)
                x0 = xt0[:, cs]
                x1 = xt1[:, cs]

                yps = ps.tile([128, CHK], F32, tag="yz")
                nc.tensor.matmul(yps[:], wsr["w1t0"][:], x0,
                                 start=True, stop=False)
                nc.tensor.matmul(yps[:], wsr["w1t1"][:], x1,
                                 start=False, stop=True)
                ay = mlp.tile([128, CHK], BF16, tag="ay")
                nc.scalar.activation(ay[:], yps[:], AF.Abs,
                                     bias=wsb["bias1"][:])

                zfull = ps.tile([128, CHK], F32, tag="zz")
                zps = zfull[0:65, :]
                nc.tensor.matmul(zps[:], wsr["a2xt0"][:], x0,
                                 start=True, stop=False)
                nc.tensor.matmul(zps[:], wsr["a2xt1"][:], x1,
                                 start=False, stop=False)
                nc.tensor.matmul(zps[:], wsr["b2xt"][:], ay[:],
                                 start=False, stop=True)
                t2 = mlp.tile([65, CHK], F32, tag="t2")
                nc.scalar.activation(t2[:], zps[:], AF.Abs,
                                     bias=wsb["bias2"][:])

                for j in range(CHK // TT):
                    cc = c * (CHK // TT) + j
                    nc.tensor.matmul(p2ps[cc // 32][:, cc % 32:cc % 32 + 1],
                                     t2[:, j * TT:(j + 1) * TT],
                                     wsb["r3"][:], start=True, stop=True)
                if c == NCHK // 2 - 1:
                    sample_block(0)
                elif c == NCHK - 1:
                    sample_block(1)

            # ================= global normalization =======================
            galr = small.tile([P, 4 * S], F32, tag="galr")
            nc.gpsimd.partition_all_reduce(galr[:], vemax[:], channels=P,
                                           reduce_op=bass_isa.ReduceOp.max)
            gmr = small.tile([P, 2], F32, tag="gmr")
            nc.vector.tensor_reduce(gmr[0:1, 0:1], galr[0:1, 0:2 * S],
                                    op=OP.max, axis=mybir.AxisListType.X)
            nc.vector.tensor_reduce(gmr[0:1, 1:2],
                                    galr[0:1, 2 * S:4 * S],
                                    op=OP.max, axis=mybir.AxisListType.X)
            ccsb = small.tile([1, 2 * NCORES], F32, tag="ccsb")
            if sim:
                gbc = small.tile([NCORES, 2], F32, tag="gbc")
                nc.gpsimd.partition_broadcast(gbc[:], gmr[0:1, 0:2],
                                              channels=NCORES)
                nc.sync.dma_start(ccsb[0:1, 0:NCORES],
                                  gbc[0:NCORES // 2, :].rearrange(
                                      "p r -> (p r)")[None, :])
                nc.sync.dma_start(ccsb[0:1, NCORES:2 * NCORES],
                                  gbc[NCORES // 2:NCORES, :].rearrange(
                                      "p r -> (p r)")[None, :])
            else:
                with tc.tile_critical():
                    cc_sem = nc.alloc_semaphore("ccs")
                    nc.gpsimd.dma_start(cc_in[:], gmr[0:1, 0:2]).then_inc(
                        cc_sem, 16)
                    nc.gpsimd.collective_compute(
                        "AllGather", OP.bypass,
                        replica_groups=[list(range(NCORES))],
                        ins=[cc_in[:]], outs=[cc_out[:]],
                    )._wait_ge(cc_sem, 16).then_inc(cc_sem, 1)
                    nc.gpsimd.dma_start(ccsb[:], cc_out[None, :])._wait_ge(
                        cc_sem, 17).then_inc(cc_sem, 16)
                    nc.gpsimd.engine_nop()._wait_ge(cc_sem, 33)

            sg = small.tile([1, 4], F32, tag="sg")
            nc.vector.tensor_reduce(sg[:, 0:1], ccsb[:, 0:2 * NCORES:2],
                                    op=OP.max, axis=mybir.AxisListType.X)
            nc.vector.tensor_reduce(sg[:, 1:2], ccsb[:, 1:2 * NCORES:2],
                                    op=OP.max, axis=mybir.AxisListType.X)
            # sigma = 0.3*emax' / (vmax + 1e-7)  (v is 10x; 0.1 folded)
            nc.vector.tensor_scalar(sg[:, 2:3], sg[:, 0:1], 1e-7, None,
                                    op0=OP.add)
            nc.vector.reciprocal_approx_fast(out=sg[:, 0:1], in_=sg[:, 2:3])
            nc.vector.tensor_scalar(sg[:, 1:2], sg[:, 1:2], 0.3, None,
                                    op0=OP.mult)
            nc.vector.tensor_tensor(sg[:, 3:4], sg[:, 0:1], sg[:, 1:2],
                                    op=OP.mult)
            sgb = small.tile([P, 1], F32, tag="sgb")
            nc.gpsimd.partition_broadcast(sgb[:], sg[0:1, 3:4], channels=P)

            for s in range(S):
                v = audio[s]["v"]
                vf = scr.tile([P, CH], F32, tag=f"envf{s}", name=f"vf{s}")
                ov = out[s].rearrange("(p n) -> p n", p=P)
                for h in range(2):
                    hs = slice(HH * h, HH * (h + 1))
                    nc.vector.tensor_scalar(vf[:, hs], v[:, hs],
                                            sgb[:, 0:1], None, op0=OP.mult)
                    nc.sync.dma_start(ov[:, hs], vf[:, hs])
    nc.finalize()
    return nc


def kernel(trace=False, **inputs):
    gru = np.ascontiguousarray(np.asarray(inputs["gru_output"], np.float32))
    enh = np.ascontiguousarray(np.asarray(inputs["enhanced"], np.float32))
    noisy = np.ascontiguousarray(np.asarray(inputs["noisy"], np.float32))
    B = gru.shape[0]
    wts = _prep_weights(inputs["W1"], inputs["b1"], inputs["a1"],
                        inputs["W2"], inputs["b2"], inputs["a2"],
                        inputs["W3"], inputs["b3"])
    m3 = _interp_m3()
    m3d = np.zeros((35, HOP), np.float32)
    m3d[0:3] = m3
    m3d[32:35] = m3
    wts["m3d"] = m3d
    wts["ident"] = np.ascontiguousarray(np.eye(128, dtype=np.float32))

    if "nc" not in _compiled:
        _compiled["nc"] = _build_nc()
    nc = _compiled["nc"]

    per = B // NCORES
    in_maps = []
    for c in range(NCORES):
        m = {
            "grut": np.ascontiguousarray(
                gru[c * per:(c + 1) * per].reshape(TB, GRU_H).T),
            "enh": np.ascontiguousarray(enh[c * per:(c + 1) * per]),
            "noisy": np.ascontiguousarray(noisy[c * per:(c + 1) * per]),
        }
        m.update(wts)
        in_maps.append(m)

    res = run_bass_kernel_spmd(nc, in_maps, list(range(NCORES)), trace=trace)
    outs = [res.results[c]["out"] for c in range(NCORES)]
    full = np.concatenate(outs, axis=0)
    if trace:
        return full, res
    return full


if __name__ == "__main__":
    pass


# revision 6
# speedup vs baseline: 1.0255x; 1.0021x over previous
"""NeuralWDRC Trainium2 kernel v2: 8-core data-parallel (2 samples/core).

Per core:
  1) MLP (baseline |x|-folded form, f32r matmuls, bf16-staged gru) -> p2
     with t on partitions: p2ps [125, 64].
  2) ratio chain: softplus via Exp/Ln (single act table set), clip [1,20].
  3) interp as ONE matmul per sample: lhsT Y [34,125] built via
     transpose -> line -> overlapped-reshape -> transpose; rhs M34 [34,2560].
  4) gain per compressor via min-1 form:
       rec' = thr/env  (recip_approx_fast of Act-Abs-scaled env)
       gain = 1 - clip((1-rec')*(1-rinv_k), 0, 0.9)
     computed in bf16 TT/TS ops (DVE 2x/4x modes), scan in bf16 with f32
     decay row and 128-col halo warmup + exact stream-start fixup.
  5) v = enh*s1 + (0.1*res)*s2 (10x scale folded into normalization),
     global abs-max via partition_all_reduce + AllGather(8).
"""

import numpy as np

import concourse.bass as bass
import concourse.bacc as bacc
import concourse.mybir as mybir
import concourse.tile as tile
from concourse.bass_utils import run_bass_kernel_spmd
from concourse import bass_isa

F32 = mybir.dt.float32
F32R = mybir.dt.float32r
BF16 = mybir.dt.bfloat16
I16 = mybir.dt.int16
AF = mybir.ActivationFunctionType
OP = mybir.AluOpType

NCORES = 8
S = 2
T = 4000
TB = S * T            # 8000
NSAMP = 320000
HOP = 80
GRU_H, H1, H2 = 256, 128, 64

P = 125               # audio partitions
CH = 2560             # cols per partition
HH = CH // 2
W = 64                # scan halo warmup cols
A = W + CH            # 2688

CHK = 500
NCHK = TB // CHK      # 16
TT = 125
LAM = 2.0 ** -10

_compiled = {}


def _prep_weights(W1, b1, a1, W2, b2, a2, W3, b3):
    W1 = W1.astype(np.float64); W2 = W2.astype(np.float64)
    w3 = W3.astype(np.float64)[2]
    b1 = b1.astype(np.float64); b2 = b2.astype(np.float64)
    b3r = float(np.asarray(b3, np.float64)[2])
    a1 = float(a1); a2 = float(a2)
    c1, d1 = (1 + a1) / 2, (1 - a1) / 2
    c2, d2 = (1 + a2) / 2, (1 - a2) / 2

    A2 = c1 * (W2 @ W1)
    B2 = d1 * W2
    beta2 = b2 + c1 * (W2 @ b1)

    a3 = c2 * (A2.T @ w3)
    b3v = c2 * (B2.T @ w3)
    c3v = d2 * w3
    gamma = c2 * float(w3 @ beta2) + b3r

    A2x = np.concatenate([A2, LAM * a3[None, :]], 0)
    B2x = np.concatenate([B2, LAM * b3v[None, :]], 0)
    beta2x = np.concatenate([beta2, [1.0]])
    r3 = np.concatenate([c3v, [1.0 / LAM]])
    spb = gamma - 1.0 / LAM

    W1T = W1.T
    out = {
        "w1t0": W1T[:128], "w1t1": W1T[128:],
        "a2xt0": A2x.T[:128], "a2xt1": A2x.T[128:],
        "b2xt": B2x.T,
        "r3": r3[:, None],
        "bias1": b1[:, None],
        "bias2": beta2x[:, None],
        "spbias": np.full((P, 1), spb),
    }
    return {k: np.ascontiguousarray(v, np.float32) for k, v in out.items()}


def _interp_m3():
    """[3, 80]: ratio_i[80t+k] = sum_j M3[j,k] * ratio[t-1+j]."""
    m = np.zeros((3, HOP), np.float64)
    for k in range(HOP):
        f = (k + 0.5) / HOP - 0.5
        if k < HOP // 2:
            m[0, k] = -f
            m[1, k] = 1.0 + f
        else:
            m[1, k] = 1.0 - f
            m[2, k] = f
    return np.ascontiguousarray(m, np.float32)


def _build_nc(sim=False, dbg=False):
    nc = bacc.Bacc("TRN2", target_bir_lowering=False, debug=False,
                   num_devices=NCORES)
    grut = nc.dram_tensor("grut", [GRU_H, TB], F32, kind="ExternalInput")
    enh = nc.dram_tensor("enh", [S, NSAMP], F32, kind="ExternalInput")
    noisy = nc.dram_tensor("noisy", [S, NSAMP], F32, kind="ExternalInput")
    wnames = ["w1t0", "w1t1", "a2xt0", "a2xt1", "b2xt", "r3",
              "bias1", "bias2", "spbias", "m3d", "ident"]
    wshapes = {"w1t0": [128, 128], "w1t1": [128, 128],
               "a2xt0": [128, 65], "a2xt1": [128, 65], "b2xt": [128, 65],
               "r3": [65, 1], "bias1": [128, 1], "bias2": [65, 1],
               "spbias": [P, 1], "m3d": [35, HOP], "ident": [128, 128]}
    wdram = {n: nc.dram_tensor(n, wshapes[n], F32, kind="ExternalInput")
             for n in wnames}
    out = nc.dram_tensor("out", [S, NSAMP], F32, kind="ExternalOutput")
    dbgd = {}
    if dbg:
        for dn, dshape in [("d_rat", [P, 32]), ("d_rinv", [P, CH]),
                           ("d_m1e", [P, CH]), ("d_m1r", [P, CH]),
                           ("d_ghe", [P, A]), ("d_sve", [P, A]),
                           ("d_ghr", [P, A]), ("d_svr", [P, A]),
                           ("d_v", [P, CH]), ("d_m2e", [P, CH]),
                           ("d_enh", [P, CH]), ("d_res", [P, CH])]:
            dbgd[dn] = nc.dram_tensor(dn, dshape, F32, kind="ExternalOutput")
    cc_in = nc.dram_tensor("cc_in", [2], F32)
    cc_out = nc.dram_tensor("cc_out", [2 * NCORES], F32, addr_space="Shared")

    with tile.TileContext(nc) as tc:
        with (
            tc.tile_pool(name="wpool", bufs=1) as wpool,
            tc.tile_pool(name="mlp", bufs=2) as mlp,
            tc.tile_pool(name="small", bufs=1) as small,
            tc.tile_pool(name="scr", bufs=1) as scr,
            tc.tile_pool(name="ps", bufs=2, space="PSUM") as ps,
            tc.tile_pool(name="ps1", bufs=1, space="PSUM") as ps1,
            tc.tile_pool(name="psi", bufs=1, space="PSUM") as psi,
            tc.tile_pool(name="psi2", bufs=2, space="PSUM") as psi2,
        ):
            # ---- resident weights ----
            wsb = {}
            for n in wnames:
                t_ = wpool.tile(wshapes[n], F32, tag=n, name=f"w_{n}")
                nc.sync.dma_start(t_[:], wdram[n][:])
                wsb[n] = t_
            wsr = {}
            for n in ("w1t0", "w1t1", "a2xt0", "a2xt1", "b2xt"):
                t_ = wpool.tile(wshapes[n], BF16, tag=n + "r", name=f"wr_{n}")
                nc.vector.tensor_copy(t_[:], wsb[n][:])
                wsr[n] = t_
            sh3 = wpool.tile([35, T + 34], BF16, tag="sh3")
            m3db = wpool.tile([35, HOP], BF16, tag="m3db")
            nc.vector.tensor_copy(m3db[:], wsb["m3d"][:])

            # decay row for scans; [0, W] = 0 is the stream-start fixup
            d0a = wpool.tile([P, A], F32, tag="d0a")
            nc.gpsimd.memset(d0a[:], 0.9)
            nc.gpsimd.memset(d0a[0:1, W:W + 1], 0.0)

            # per-partition act scale/bias constants
            scl = wpool.tile([128, 2], F32, tag="scl")
            nc.gpsimd.memset(scl[:, 0:1], 1.0 / 0.3)    # enh: 1/thr
            nc.gpsimd.memset(scl[:, 1:2], 100.0)        # res': 1/0.01
            epsb = wpool.tile([128, 2], F32, tag="epsb")
            nc.gpsimd.memset(epsb[:, 0:1], 1e-8 / 0.3)
            nc.gpsimd.memset(epsb[:, 1:2], 1e-7)

            p2ps = [ps1.tile([P, 32], F32, tag="p2", name=f"p2_{i}")
                    for i in range(S)]

            def pe_touch(ap):
                pass

            vemax = small.tile([P, 4 * S], F32, tag="vemax")

            # ================= gru staging (host-transposed, bf16) ========
            xt0 = wpool.tile([128, TB], BF16, tag="xt0")
            xt1 = wpool.tile([128, TB], BF16, tag="xt1")
            def gruq(q):
                qs = slice(q * (TB // 4), (q + 1) * (TB // 4))
                nc.gpsimd.dma_start(xt0[:, qs], grut[0:128, qs])
                nc.gpsimd.dma_start(xt1[:, qs], grut[128:256, qs])
            gruq(0)

            # ================= audio pre-ratio (no MLP dependency) ========
            audio = []   # per sample: dict of tiles
            for s in range(S):
                enh_t = scr.tile([P, CH], BF16, tag=f"enh{s}", name=f"enh{s}")
                res_t = scr.tile([P, CH], BF16, tag=f"res{s}", name=f"res{s}")
                nc.gpsimd.dma_start(enh_t[:],
                                    enh[s].rearrange("(p n) -> p n", p=P))
                nc.gpsimd.dma_start(res_t[:],
                                    noisy[s].rearrange("(p n) -> p n", p=P))
                if s == 0:
                    gruq(1)
                else:
                    gruq(2)
                    gruq(3)
                # res' = 0.1 * (noisy - enh)   (thr scales to 0.01)
                nc.vector.tensor_tensor(res_t[:], res_t[:], enh_t[:],
                                        op=OP.subtract)
                nc.vector.tensor_scalar(res_t[:], res_t[:], 0.1, None,
                                        op0=OP.mult)
                audio.append({"enh": enh_t, "res": res_t})

            # env -> rec -> m1 chains
            def env_chains(s):
                au = audio[s]
                for ci, (sig, sci) in enumerate(((au["enh"], 0),
                                                 (au["res"], 1))):
                    m1 = scr.tile([P, CH], BF16, tag=f"m1_{s}{ci}",
                                  name=f"m1_{s}{ci}")
                    # env' = |x/thr + eps'| (eps folded into bias; no bf16
                    # input can cancel it to 0 exactly)
                    envf = scr.tile([P, CH], F32, tag=f"envf{ci}",
                                    name=f"envf{ci}_{s}")
                    nc.scalar.activation(envf[:], sig[:], AF.Abs,
                                         scale=scl[:P, sci:sci + 1],
                                         bias=epsb[:P, sci:sci + 1])
                    if ci == 0:
                        # emax' = max(env') ; true emax = 0.3 * emax'
                        nc.vector.tensor_reduce(
                            vemax[:, 2 * S + 2 * s:2 * S + 2 * s + 1],
                            envf[:, 0:HH],
                            op=OP.max, axis=mybir.AxisListType.X)
                        nc.vector.tensor_reduce(
                            vemax[:, 2 * S + 2 * s + 1:2 * S + 2 * s + 2],
                            envf[:, HH:CH],
                            op=OP.max, axis=mybir.AxisListType.X)
                    recf = scr.tile([P, CH], F32, tag=f"recf{ci}",
                                    name=f"recf{ci}_{s}")
                    nc.vector.reciprocal_approx_fast(out=recf[:],
                                                     in_=envf[:])
                    # m1 = relu(1 - rec')   (bf16)
                    nc.scalar.activation(m1[:], recf[:], AF.Relu,
                                         bias=1.0, scale=-1.0)
                    au[f"m1_{ci}"] = m1

            env_chains(0)
            env_chains(1)

            # ================= post-ratio audio ===========================
            def sample_block(s):
                au = audio[s]
                # ---- ratio chain [125, 32] ----
                # softplus(x) = relu(x) + poly(e^-|x|); Exp/Abs/Relu all
                # live in act-table set 0 -> no table switches
                px = small.tile([P, 32], F32, tag=f"px{s}", name=f"px{s}")
                nc.vector.tensor_copy(px[:], p2ps[s][:])
                ax = small.tile([P, 32], F32, tag=f"ax{s}", name=f"ax{s}")
                nc.scalar.activation(ax[:], px[:], AF.Abs,
                                     bias=wsb["spbias"][:])
                uu = small.tile([P, 32], F32, tag=f"uu{s}", name=f"uu{s}")
                nc.scalar.activation(uu[:], ax[:], AF.Exp, scale=-1.0)
                hh = small.tile([P, 32], F32, tag=f"hh{s}", name=f"hh{s}")
                nc.vector.tensor_scalar(hh[:], uu[:], 0.11477816,
                                        -0.40741059, op0=OP.mult, op1=OP.add)
                nc.vector.tensor_tensor(hh[:], hh[:], uu[:], op=OP.mult)
                nc.vector.tensor_scalar(hh[:], hh[:], 0.98669098, None,
                                        op0=OP.add)
                nc.vector.tensor_tensor(hh[:], hh[:], uu[:], op=OP.mult)
                rat = small.tile([P, 32], F32, tag=f"rat{s}", name=f"rat{s}")
                nc.vector.tensor_scalar(rat[:], px[:],
                                        wsb["spbias"][:P, 0:1], 0.0,
                                        op0=OP.add, op1=OP.max)
                nc.vector.tensor_tensor(rat[:], rat[:], hh[:], op=OP.add)
                nc.vector.tensor_scalar(rat[:], rat[:], 1.0, 20.0,
                                        op0=OP.add, op1=OP.min)
                if dbg and s == 0:
                    nc.sync.dma_start(dbgd["d_rat"][:], rat[:])
                # ---- sh3 rows (baseline scheme): row b+j col i =
                # ratio_s(i + j - 1), clipped at stream edges ----
                ratT_ps = psi.tile([32, P], F32, tag="rT")
                pe_touch(rat)
                nc.tensor.transpose(ratT_ps[:], rat[:], wsb["ident"][:P, :P])
                ratT = small.tile([32, P], BF16, tag=f"ratT{s}",
                                  name=f"ratT{s}")
                nc.scalar.copy(ratT[:], ratT_ps[:])
                b = 32 * s
                rT = ratT[:]
                r3d = lambda ap: ap.rearrange("p (r q) -> p r q", q=P)
                nc.sync.dma_start(r3d(sh3[b:b + 1, 1:T + 1]), rT)
                nc.sync.dma_start(sh3[b:b + 1, 0:1], rT[0:1, 0:1])
                nc.sync.dma_start(r3d(sh3[b + 1:b + 2, 0:T]), rT)
                nc.sync.dma_start(sh3[b + 2:b + 3, 0:124], rT[0:1, 1:P])
                nc.sync.dma_start(
                    r3d(sh3[b + 2:b + 3, 124:124 + 31 * P]), rT[1:32, :])
                nc.sync.dma_start(sh3[b + 2:b + 3, T - 1:T],
                                  rT[31:32, P - 1:P])
                # ---- interp (group matmuls) + rinv from psum ----
                rinv = scr.tile([P, CH], F32, tag="rcr", name=f"rinv{s}")
                pe_touch(sh3[0:1, 0:1] if s == 0 else sh3[0:1, 1:2])
                for g in range(6):
                    taus = list(range(g * 6, min((g + 1) * 6, 32)))
                    rips = psi2.tile([P, 480], F32, tag="rips")
                    for ti, tau in enumerate(taus):
                        lhsT = sh3[b:b + 3, tau:tau + 32 * P:32]
                        nc.tensor.matmul(rips[:, ti * HOP:(ti + 1) * HOP],
                                         lhsT, m3db[b:b + 3, :],
                                         start=True, stop=True)
                    nwid = len(taus) * HOP
                    nc.vector.reciprocal_approx_fast(
                        out=rinv[:, g * 480:g * 480 + nwid],
                        in_=rips[:, :nwid])
                if dbg and s == 0:
                    nc.sync.dma_start(dbgd["d_rinv"][:], rinv[:])
                m2e = scr.tile([P, CH], BF16, tag=f"m2e{s}", name=f"m2e{s}")
                m2r = scr.tile([P, CH], BF16, tag=f"m2r{s}", name=f"m2r{s}")
                for h in range(2):
                    hs = slice(HH * h, HH * (h + 1))
                    nc.scalar.activation(m2e[:, hs], rinv[:, hs], AF.Copy,
                                         bias=1.0, scale=-1.0)
                    nc.scalar.activation(m2r[:, hs], m2e[:, hs], AF.Copy,
                                         bias=-1.0, scale=2.0)

                if dbg and s == 0:
                    nc.gpsimd.dma_start(dbgd["d_m2e"][:], m2e[:])
                    nc.gpsimd.dma_start(dbgd["d_m1e"][:], au["m1_0"][:])
                    nc.gpsimd.dma_start(dbgd["d_m1r"][:], au["m1_1"][:])
                    nc.gpsimd.dma_start(dbgd["d_enh"][:], au["enh"][:])
                    nc.gpsimd.dma_start(dbgd["d_res"][:], au["res"][:])
                # ---- gains + scans + combine ----
                svs = []
                for ci, m2 in enumerate((m2e, m2r)):
                    m1 = au[f"m1_{ci}"]
                    gh = scr.tile([P, A], BF16, tag=f"gh{ci}",
                                  name=f"gh{s}{ci}")
                    for h in range(2):
                        hs = slice(HH * h, HH * (h + 1))
                        gmh = gh[:, W + HH * h:W + HH * (h + 1)]
                        nc.vector.tensor_tensor(gmh, m1[:, hs], m2[:, hs],
                                                op=OP.mult)
                        nc.vector.tensor_scalar(gmh, gmh, -1.0, 0.9,
                                                op0=OP.max, op1=OP.min)
                        nc.scalar.activation(gmh, gmh, AF.Copy,
                                             bias=1.0, scale=-1.0)
                    # halo: prev partition's last W gains
                    ght = scr.tile([P, W], BF16, tag=f"ght{ci}",
                                   name=f"ght{s}{ci}")
                    nc.vector.tensor_copy(ght[:], gh[:, A - W:A])
                    nc.sync.dma_start(gh[1:P, 0:W], ght[0:P - 1, :])
                    nc.gpsimd.memset(gh[0:1, 0:W], 1.0)
                    # stream-start fixup: s[0, W] must be 10*g0
                    nc.vector.tensor_scalar(gh[0:1, W:W + 1],
                                            gh[0:1, W:W + 1], 10.0, None,
                                            op0=OP.mult)
                    sv = scr.tile([P, A], BF16, tag=f"sv{ci}",
                                  name=f"sv{s}{ci}")
                    nc.vector.tensor_tensor_scan(sv[:], d0a[:], gh[:], 0.0,
                                                 op0=OP.mult, op1=OP.add)
                    if dbg and s == 0:
                        nm = "e" if ci == 0 else "r"
                        nc.gpsimd.dma_start(dbgd["d_gh" + nm][:], gh[:])
                        nc.gpsimd.dma_start(dbgd["d_sv" + nm][:], sv[:])
                    svs.append(sv)

                # v = enh*s1 + res'*s2 (both scans carry 10x; folded later)
                ce = scr.tile([P, CH], BF16, tag=f"ce{s}", name=f"ce{s}")
                nc.vector.tensor_tensor(ce[:], au["enh"][:], svs[0][:, W:A],
                                        op=OP.mult)
                cr = scr.tile([P, CH], BF16, tag="rcr", name=f"cr{s}")
                nc.vector.tensor_tensor(cr[:], au["res"][:], svs[1][:, W:A],
                                        op=OP.mult)
                nc.vector.tensor_tensor(ce[:], ce[:], cr[:], op=OP.add)
                for h in range(2):
                    hs = slice(HH * h, HH * (h + 1))
                    nc.vector.tensor_reduce(
                        vemax[:, 2 * s + h:2 * s + h + 1], ce[:, hs],
                        op=OP.max, axis=mybir.AxisListType.X,
                        apply_absolute_value=True)
                if dbg and s == 0:
                    nc.gpsimd.dma_start(dbgd["d_v"][:], ce[:])
                au["v"] = ce

            # ================= MLP chunks =================================
            for c in range(NCHK):
                cs = slice(c * CHK, (c + 1) * CHK)
                x0 = xt0[:, cs]
                x1 = xt1[:, cs]

                yps = ps.tile([128, CHK], F32, tag="yz")
                nc.tensor.matmul(yps[:], wsr["w1t0"][:], x0,
                                 start=True, stop=False)
                nc.tensor.matmul(yps[:], wsr["w1t1"][:], x1,
                                 start=False, stop=True)
                ay = mlp.tile([128, CHK], BF16, tag="ay")
                nc.scalar.activation(ay[:], yps[:], AF.Abs,
                                     bias=wsb["bias1"][:])

                zfull = ps.tile([128, CHK], F32, tag="zz")
                zps = zfull[0:65, :]
                nc.tensor.matmul(zps[:], wsr["a2xt0"][:], x0,
                                 start=True, stop=False)
                nc.tensor.matmul(zps[:], wsr["a2xt1"][:], x1,
                                 start=False, stop=False)
                nc.tensor.matmul(zps[:], wsr["b2xt"][:], ay[:],
                                 start=False, stop=True)
                t2 = mlp.tile([65, CHK], F32, tag="t2")
                nc.scalar.activation(t2[:], zps[:], AF.Abs,
                                     bias=wsb["bias2"][:])

                for j in range(CHK // TT):
                    cc = c * (CHK // TT) + j
                    nc.tensor.matmul(p2ps[cc // 32][:, cc % 32:cc % 32 + 1],
                                     t2[:, j * TT:(j + 1) * TT],
                                     wsb["r3"][:], start=True, stop=True)
                if c == NCHK // 2 - 1:
                    sample_block(0)
                elif c == NCHK - 1:
                    sample_block(1)

            # ================= global normalization =======================
            galr = small.tile([P, 4 * S], F32, tag="galr")
            nc.gpsimd.partition_all_reduce(galr[:], vemax[:], channels=P,
                                           reduce_op=bass_isa.ReduceOp.max)
            gmr = small.tile([P, 2], F32, tag="gmr")
            nc.vector.tensor_reduce(gmr[0:1, 0:1], galr[0:1, 0:2 * S],
                                    op=OP.max, axis=mybir.AxisListType.X)
            nc.vector.tensor_reduce(gmr[0:1, 1:2],
                                    galr[0:1, 2 * S:4 * S],
                                    op=OP.max, axis=mybir.AxisListType.X)
            ccsb = small.tile([1, 2 * NCORES], F32, tag="ccsb")
            if sim:
                gbc = small.tile([NCORES, 2], F32, tag="gbc")
                nc.gpsimd.partition_broadcast(gbc[:], gmr[0:1, 0:2],
                                              channels=NCORES)
                nc.sync.dma_start(ccsb[0:1, 0:NCORES],
                                  gbc[0:NCORES // 2, :].rearrange(
                                      "p r -> (p r)")[None, :])
                nc.sync.dma_start(ccsb[0:1, NCORES:2 * NCORES],
                                  gbc[NCORES // 2:NCORES, :].rearrange(
                                      "p r -> (p r)")[None, :])
            else:
                with tc.tile_critical():
                    cc_sem = nc.alloc_semaphore("ccs")
                    nc.gpsimd.dma_start(cc_in[:], gmr[0:1, 0:2]).then_inc(
                        cc_sem, 16)
                    nc.gpsimd.collective_compute(
                        "AllGather", OP.bypass,
                        replica_groups=[list(range(NCORES))],
                        ins=[cc_in[:]], outs=[cc_out[:]],
                    )._wait_ge(cc_sem, 16).then_inc(cc_sem, 1)
                    nc.gpsimd.dma_start(ccsb[:], cc_out[None, :])._wait_ge(
                        cc_sem, 17).then_inc(cc_sem, 16)
                    nc.gpsimd.engine_nop()._wait_ge(cc_sem, 33)

            sg = small.tile([1, 4], F32, tag="sg")
            nc.vector.tensor_reduce(sg[:, 0:1], ccsb[:, 0:2 * NCORES:2],
                                    op=OP.max, axis=mybir.AxisListType.X)
            nc.vector.tensor_reduce(sg[:, 1:2], ccsb[:, 1:2 * NCORES:2],
                                    op=OP.max, axis=mybir.AxisListType.X)
            # sigma = 0.3*emax' / (vmax + 1e-7)  (v is 10x; 0.1 folded)
            nc.vector.tensor_scalar(sg[:, 2:3], sg[:, 0:1], 1e-7, None,
                                    op0=OP.add)
            nc.vector.reciprocal_approx_fast(out=sg[:, 0:1], in_=sg[:, 2:3])
            nc.vector.tensor_scalar(sg[:, 1:2], sg[:, 1:2], 0.3, None,
                                    op0=OP.mult)
            nc.vector.tensor_tensor(sg[:, 3:4], sg[:, 0:1], sg[:, 1:2],
                                    op=OP.mult)
            sgb = small.tile([P, 1], F32, tag="sgb")
            nc.gpsimd.partition_broadcast(sgb[:], sg[0:1, 3:4], channels=P)

            for s in range(S):
                v = audio[s]["v"]
                vf = scr.tile([P, CH], F32, tag=f"envf{s}", name=f"vf{s}")
                ov = out[s].rearrange("(p n) -> p n", p=P)
                for h in range(2):
                    hs = slice(HH * h, HH * (h + 1))
                    nc.vector.tensor_scalar(vf[:, hs], v[:, hs],
                                            sgb[:, 0:1], None, op0=OP.mult)
                    nc.sync.dma_start(ov[:, hs], vf[:, hs])
    nc.finalize()
    return nc


def kernel(trace=False, **inputs):
    gru = np.ascontiguousarray(np.asarray(inputs["gru_output"], np.float32))
    enh = np.ascontiguousarray(np.asarray(inputs["enhanced"], np.float32))
    noisy = np.ascontiguousarray(np.asarray(inputs["noisy"], np.float32))
    B = gru.shape[0]
    wts = _prep_weights(inputs["W1"], inputs["b1"], inputs["a1"],
                        inputs["W2"], inputs["b2"], inputs["a2"],
                        inputs["W3"], inputs["b3"])
    m3 = _interp_m3()
    m3d = np.zeros((35, HOP), np.float32)
    m3d[0:3] = m3
    m3d[32:35] = m3
    wts["m3d"] = m3d
    wts["ident"] = np.ascontiguousarray(np.eye(128, dtype=np.float32))

    if "nc" not in _compiled:
        _compiled["nc"] = _build_nc()
    nc = _compiled["nc"]

    per = B // NCORES
    in_maps = []
    for c in range(NCORES):
        m = {
            "grut": np.ascontiguousarray(
                gru[c * per:(c + 1) * per].reshape(TB, GRU_H).T),
            "enh": np.ascontiguousarray(enh[c * per:(c + 1) * per]),
            "noisy": np.ascontiguousarray(noisy[c * per:(c + 1) * per]),
        }
        m.update(wts)
        in_maps.append(m)

    res = run_bass_kernel_spmd(nc, in_maps, list(range(NCORES)), trace=trace)
    outs = [res.results[c]["out"] for c in range(NCORES)]
    full = np.concatenate(outs, axis=0)
    if trace:
        return full, res
    return full


if __name__ == "__main__":
    pass
